# revision 21
# baseline (speedup 1.0000x reference)
"""GCLayer (GNN message passing) on 8 Trainium2 NeuronCores.

Strategy
--------
Edges are partitioned by destination row (node-range sharding): core c owns
nodes [c*6250, (c+1)*6250) and every edge whose `row` lands there, so the
segment-sum needs no cross-core reduction.

Phase 1 (node stage, sharded): x = h@lin_w+b, x_ = msg_mlp(x),
A = x@att_w1[:D]+att_b1 per local node (row-major).  The host assembles the
global x / x_ tables (bf16) and feeds them to every core.

Phase 2 (edge + out stage), feature-major ("flipped") attention:
  hid^T[j, e] = A^T-gather (aloc lhsT @ one-hot rhs) + wc lhsT @ x^T[col]
                + we outer ea           (PSUM, blocks up to 512 edges wide)
  sil^T = SILU(hid^T)                   (ACT, one op per block)
  logit[e] = sil^T slice (lhsT) @ w2col (PE does the w2 dot, one col/tile)
  att = sigmoid(logits + b2)            (ACT, ONE op per 64-tile chunk ->
                                         no per-tile SILU/SIGMOID table churn)
  sel[e, s] = att_e * (iota_f == lrow_e) (vector tensor_scalar, fused)
  msg^T[d, s] += xm[col] (lhsT) @ sel   (PE scatter, transposed msg)
Masking/padding is exact via a sentinel lrow (=200) so sel columns of pad or
masked edges are all-zero; no -30000 bias hack needed.

x^T[col] / xm[col] are fetched with SWDGE dma_gather in prepare_only mode +
trigger_dma on 4 SWDGE queues, so the gpsimd engine only pays descriptor
generation and the random-access HBM transfers overlap compute.

LayerNorms (phase 1, out stage) are two-pass with batched stats: SILU keeps
its ACT table resident; Var = E[x^2]-mu^2 via scalar_tensor_tensor accum;
sqrt over all 49 windows in one ACT op.
"""

import sys

sys.path.insert(0, "/opt/trn_rl_repo")

import numpy as np
import ml_dtypes

from concourse import bacc, mybir, tile
from concourse.bass_utils import run_bass_kernel_spmd

BF16 = ml_dtypes.bfloat16
F32 = np.float32

NCORES = 8
N = 50000
E = 800000
D = 128
NL = N // NCORES          # 6250 real nodes per core
NW = 49                   # node blocks of 128 per core (49*128 = 6272)
NLP = NW * 128            # padded nodes per core
SPLIT = 32768             # int16 gather index limit
CHUNK = 8192              # edges per dma_gather call (64 tiles)
RCH = 2048                # row-stream load granularity (16 tiles)
BLKT = 4                  # tiles per hidden block (<=512 edges)
PREP_GATHER = True        # prepare_only + trigger_dma (overlapped transfers)
LN_EPS = 1e-5
SENT = 200.0              # sentinel lrow for pad/masked edges (> 127)

FP = mybir.dt.float32
BF = mybir.dt.bfloat16
I16 = mybir.dt.int16

TRACE = False             # test.py sets kernel.TRACE = True for profiling
LAST_RESULTS = {}         # exec_time_ns per phase when TRACE

_cache = {}


def _bf(a):
    return np.ascontiguousarray(np.asarray(a, dtype=F32).astype(BF16))


def _f32(a):
    return np.ascontiguousarray(np.asarray(a, dtype=F32))


def _ceil(a, m):
    return -(-int(a) // m) * m


# ---------------------------------------------------------------------------
# Host-side edge preprocessing
# ---------------------------------------------------------------------------

def _prep_edges(row, col, ea, em):
    """Sort/partition/pad edges. Returns per-core streams + the static plan."""
    row = np.asarray(row).astype(np.int64).ravel()
    col = np.asarray(col).astype(np.int64).ravel()
    ea = np.asarray(ea, dtype=F32).ravel()
    em = np.asarray(em, dtype=F32).ravel()

    c_of = row // NL
    r_loc = row - c_of * NL
    w_of = r_loc // 128
    s_in_w = r_loc % 128
    hi = (col >= SPLIT).astype(np.int64)

    # bucket = (core, pass(lo/hi), window); argsort gives the stream order
    key = (c_of * 2 + hi) * NW + w_of
    order = np.argsort(key, kind="stable")
    skey = key[order]
    cnt = np.bincount(key, minlength=NCORES * 2 * NW).reshape(NCORES, 2, NW)

    WL = np.array([_ceil(cnt[:, 0, w].max(), 128) for w in range(NW)])
    WH = np.array([_ceil(cnt[:, 1, w].max(), 128) for w in range(NW)])
    lo_total = int(WL.sum())
    hi_total = int(WH.sum())
    EP = lo_total + hi_total

    # padded base offset of each (pass, window) block within the stream
    sizes = np.concatenate([WL, WH])                      # (2*NW,)
    base = np.concatenate([[0], np.cumsum(sizes)[:-1]])   # (2*NW,)

    # rank of each edge within its bucket
    bstart = np.concatenate([[0], np.cumsum(cnt.ravel())[:-1]])
    rank = np.arange(row.size) - bstart[skey]
    pw = skey % (2 * NW)                                   # (pass, window) id
    dest = base[pw] + rank                                 # position in stream
    cc = skey // (2 * NW)

    g_lrow = np.zeros((NCORES, EP), F32)
    g_sel = np.full((NCORES, EP), SENT, F32)   # sentinel: pad edges -> sel 0
    g_idx = np.zeros((NCORES, EP), np.int16)
    g_ea = np.zeros((NCORES, EP), F32)

    e_ids = order
    g_lrow[cc, dest] = s_in_w[e_ids].astype(F32)
    # masked real edges keep the sentinel too (exact edge_mask handling)
    lsel = np.where(em[e_ids] > 0.5, s_in_w[e_ids].astype(F32), SENT)
    g_sel[cc, dest] = lsel
    g_idx[cc, dest] = (col[e_ids] - hi[e_ids] * SPLIT).astype(np.int16)
    g_ea[cc, dest] = ea[e_ids]

    # wrapped layouts
    sel_col = np.ascontiguousarray(
        g_sel.reshape(NCORES, EP // 128, 128).transpose(0, 2, 1))
    idx16 = g_idx.reshape(NCORES, EP // 16, 16).transpose(0, 2, 1)  # (8,16,EP/16)
    idx_w = np.ascontiguousarray(np.tile(idx16, (1, 8, 1)))         # (8,128,EP/16)
    lrow_row = _bf(g_lrow.reshape(NCORES, 1, EP))
    ea_row = _bf(g_ea.reshape(NCORES, 1, EP))

    def chunks(total, start):
        out = []
        off = 0
        while off < total:
            g = min(CHUNK, total - off)
            out.append((start + off, g))
            off += g
        return out

    plan = dict(
        WL=[int(x) for x in WL], WH=[int(x) for x in WH], EP=EP,
        lo_total=lo_total, hi_total=hi_total,
        chunks_lo=chunks(lo_total, 0), chunks_hi=chunks(hi_total, lo_total),
    )
    streams = dict(sel_col=sel_col, idx=idx_w,
                   lrow_row=lrow_row, ea_row=ea_row)
    return plan, streams


# ---------------------------------------------------------------------------
# Phase 1: node stage (sharded over nodes)
# ---------------------------------------------------------------------------

def _build_phase1():
    nc = bacc.Bacc("TRN2", target_bir_lowering=False, debug=False,
                   num_devices=NCORES)
    g = lambda n, s, d, k: nc.dram_tensor(n, s, d, kind=k).ap()

    ht = g("ht", [128, NLP], BF, "ExternalInput")          # h^T, node-wrapped
    linw = g("linw", [128, 128], BF, "ExternalInput")
    linb = g("linb_row", [1, 128], BF, "ExternalInput")
    w1m = g("w1m", [128, 128], BF, "ExternalInput")
    b1m = g("b1m_row", [1, 128], BF, "ExternalInput")
    w2mp = g("w2mp", [128, 128], BF, "ExternalInput")
    b2mp = g("b2mp_row", [1, 128], BF, "ExternalInput")
    wr = g("wr", [128, 128], BF, "ExternalInput")
    b1a = g("b1a_row", [1, 128], BF, "ExternalInput")
    ident = g("ident", [128, 128], BF, "ExternalInput")
    ones1 = g("ones1", [1, 128], BF, "ExternalInput")

    eps_c = g("eps_col", [128, 1], FP, "ExternalInput")

    x_out = g("x_out", [128, NLP], FP, "ExternalOutput")
    xm_out = g("xm_out", [128, NLP], BF, "ExternalOutput")
    a_out = g("a_out", [128, NLP], BF, "ExternalOutput")

    SILU = mybir.ActivationFunctionType.Silu
    SQRT = mybir.ActivationFunctionType.Sqrt
    MUL = mybir.AluOpType.mult
    SUB = mybir.AluOpType.subtract
    BYP = mybir.AluOpType.bypass

    with tile.TileContext(nc) as tc:
        with tc.tile_pool(name="const", bufs=1) as cp, \
             tc.tile_pool(name="work", bufs=3) as wp, \
             tc.tile_pool(name="psum", bufs=1, space="PSUM") as pp, \
             tc.tile_pool(name="psum2", bufs=2, space="PSUM") as pp2:

            def cload(ap, shape, dt, tag):
                t = cp.tile(shape, dt, tag=tag)
                nc.sync.dma_start(t[:], ap)
                return t

            ht_s = cload(ht, [128, NLP], BF, "c_ht")
            linw_s = cload(linw, [128, 128], BF, "c_linw")
            linb_s = cload(linb, [1, 128], BF, "c_linb")
            w1m_s = cload(w1m, [128, 128], BF, "c_w1m")
            b1m_s = cload(b1m, [1, 128], BF, "c_b1m")
            w2mp_s = cload(w2mp, [128, 128], BF, "c_w2mp")
            b2mp_s = cload(b2mp, [1, 128], BF, "c_b2mp")
            wr_s = cload(wr, [128, 128], BF, "c_wr")
            b1a_s = cload(b1a, [1, 128], BF, "c_b1a")
            id_s = cload(ident, [128, 128], BF, "c_id")
            ones_s = cload(ones1, [1, 128], BF, "c_ones")
            eps_s = cload(eps_c, [128, 1], FP, "c_eps")

            s1buf = cp.tile([128, NLP], FP, tag="s1buf")
            musum = cp.tile([128, NW], FP, tag="musum")
            sqsum = cp.tile([128, NW], FP, tag="sqsum")
            xtbuf = cp.tile([128, NLP], BF, tag="xtbuf")

            # ---- pass A: x, x^T, A, s1 = silu(mlp1(x)), LN stats ---------
            for w in range(NW):
                blk = slice(w * 128, (w + 1) * 128)
                htb = ht_s[:, blk]

                # x = h @ lin_w + lin_b      [n, d]  (row-major)
                px = pp.tile([128, 128], FP, tag="px")
                nc.tensor.matmul(px[:], htb, linw_s[:], start=True, stop=False)
                nc.tensor.matmul(px[:], ones_s[:], linb_s[:], start=False,
                                 stop=True)
                xs = wp.tile([128, 128], FP, tag="xs")
                nc.scalar.copy(xs[:], px[:])
                nc.sync.dma_start(x_out[:, blk], xs[:])

                # x^T  (for downstream lhsT use)
                pxt = pp.tile([128, 128], FP, tag="pxt")
                nc.tensor.matmul(pxt[:], linw_s[:], htb, start=True, stop=False)
                nc.tensor.matmul(pxt[:], linb_s[:], ones_s[:], start=False,
                                 stop=True)
                nc.vector.tensor_copy(xtbuf[:, blk], pxt[:])

                # A = x @ wr + b1a           [n, h]  (row-major)
                pa = pp.tile([128, 128], FP, tag="pa")
                nc.tensor.matmul(pa[:], xtbuf[:, blk], wr_s[:], start=True,
                                 stop=False)
                nc.tensor.matmul(pa[:], ones_s[:], b1a_s[:], start=False,
                                 stop=True)
                asb = wp.tile([128, 128], BF, tag="asb")
                nc.scalar.copy(asb[:], pa[:])
                nc.sync.dma_start(a_out[:, blk], asb[:])

                # s1 = silu(x @ w1m + b1m), musum = sum(s1)
                ps = pp.tile([128, 128], FP, tag="ps")
                nc.tensor.matmul(ps[:], xtbuf[:, blk], w1m_s[:], start=True,
                                 stop=False)
                nc.tensor.matmul(ps[:], ones_s[:], b1m_s[:], start=False,
                                 stop=True)
                nc.scalar.activation(s1buf[:, blk], ps[:], SILU,
                                     accum_out=musum[:, w:w + 1])
                jk = wp.tile([128, 128], BF, tag="jk")
                nc.vector.scalar_tensor_tensor(
                    jk[:], s1buf[:, blk], 0.0, s1buf[:, blk], BYP, MUL,
                    accum_out=sqsum[:, w:w + 1])

            # ---- batched LN stats ---------------------------------------
            mu = cp.tile([128, NW], FP, tag="mu")
            nc.vector.tensor_scalar(mu[:], musum[:], 1.0 / 128.0, None, MUL)
            musq = wp.tile([128, NW], FP, tag="musq")
            nc.vector.tensor_tensor(musq[:], mu[:], mu[:], MUL)
            var = wp.tile([128, NW], FP, tag="var")
            nc.vector.tensor_scalar(var[:], sqsum[:], 1.0 / 128.0, None, MUL)
            var2 = wp.tile([128, NW], FP, tag="var2")
            nc.vector.tensor_tensor(var2[:], var[:], musq[:], SUB)
            std = wp.tile([128, NW], FP, tag="std")
            nc.scalar.activation(std[:], var2[:], SQRT, bias=eps_s[:, 0:1])
            rstd = cp.tile([128, NW], FP, tag="rstd")
            nc.vector.reciprocal(rstd[:], std[:])

            # ---- pass B: z -> x_ = z @ w2mp + b2mp ----------------------
            for w in range(NW):
                blk = slice(w * 128, (w + 1) * 128)
                z = wp.tile([128, 128], BF, tag="z")
                nc.vector.tensor_scalar(z[:], s1buf[:, blk], mu[:, w:w + 1],
                                        rstd[:, w:w + 1], SUB, MUL)
                pzt = pp2.tile([128, 128], BF, tag="pzt")
                nc.tensor.transpose(pzt[:], z[:], id_s[:])
                zt = wp.tile([128, 128], BF, tag="zt")
                nc.vector.tensor_copy(zt[:], pzt[:])
                pxm = pp.tile([128, 128], FP, tag="pxm")
                nc.tensor.matmul(pxm[:], zt[:], w2mp_s[:], start=True,
                                 stop=False)
                nc.tensor.matmul(pxm[:], ones_s[:], b2mp_s[:], start=False,
                                 stop=True)
                xm = wp.tile([128, 128], BF, tag="xm")
                nc.scalar.copy(xm[:], pxm[:])
                nc.sync.dma_start(xm_out[:, blk], xm[:])

    nc.compile()
    return nc


def _phase1_inputs(h, weights):
    """Per-core in_maps for phase 1."""
    (lin_w, lin_b, msg_w1, msg_b1, msg_ln_g, msg_ln_b, msg_w2, msg_b2,
     att_w1, att_b1) = weights
    w2mp = msg_ln_g[:, None] * msg_w2
    b2mp = msg_ln_b @ msg_w2 + msg_b2
    consts = {
        "linw": _bf(lin_w), "linb_row": _bf(lin_b[None, :]),
        "w1m": _bf(msg_w1), "b1m_row": _bf(msg_b1[None, :]),
        "w2mp": _bf(w2mp), "b2mp_row": _bf(b2mp[None, :]),
        "wr": _bf(att_w1[0:D]), "b1a_row": _bf(att_b1[None, :]),
        "ident": _bf(np.eye(128)), "ones1": _bf(np.ones((1, 128))),
        "eps_col": _f32(np.full((128, 1), LN_EPS)),
    }
    in_maps = []
    for c in range(NCORES):
        hc = np.zeros((NLP, D), F32)
        hc[:NL] = h[c * NL:(c + 1) * NL]
        in_maps.append({"ht": _bf(hc.T), **consts})
    return in_maps


def _run_phase1(h, weights):
    if "p1" not in _cache:
        _cache["p1"] = _build_phase1()
    nc = _cache["p1"]
    in_maps = _phase1_inputs(h, weights)
    res = run_bass_kernel_spmd(nc, in_maps, core_ids=list(range(NCORES)),
                               trace=TRACE)
    if TRACE:
        LAST_RESULTS["phase1_ns"] = res.exec_time_ns
    return res.results


# ---------------------------------------------------------------------------
# Phase 2: edge stage (gather/attention/scatter) + out stage
# ---------------------------------------------------------------------------

def _build_phase2(plan):
    WL, WH, EP = plan["WL"], plan["WH"], plan["EP"]
    chunks_lo, chunks_hi = plan["chunks_lo"], plan["chunks_hi"]
    lo_tiles = plan["lo_total"] // 128

    nc = bacc.Bacc("TRN2", target_bir_lowering=False, debug=False,
                   num_devices=NCORES, num_swdge_queues=4)
    g = lambda n, s, d, k: nc.dram_tensor(n, s, d, kind=k).ap()

    xtab = g("xtab", [N, 128], BF, "ExternalInput")
    xmtab = g("xmtab", [N, 128], BF, "ExternalInput")
    aloc = g("aloc", [128, NLP], BF, "ExternalInput")      # A rows per window
    xres = g("xres", [128, NLP], FP, "ExternalInput")      # x rows per window
    selc = g("sel_col", [128, EP // 128], FP, "ExternalInput")
    idxt = g("idx", [128, EP // 16], I16, "ExternalInput")
    lrowr = g("lrow_row", [1, EP], BF, "ExternalInput")
    ear = g("ea_row", [1, EP], BF, "ExternalInput")

    iota_p = g("iota_part", [128, 512], FP, "ExternalInput")
    iota_f = g("iota_free", [128, 128], FP, "ExternalInput")
    wc = g("wc", [128, 128], BF, "ExternalInput")
    we = g("we_row", [1, 128], BF, "ExternalInput")
    w2c = g("w2col", [128, 1], BF, "ExternalInput")
    ident = g("ident", [128, 128], BF, "ExternalInput")
    ones1 = g("ones1", [1, 128], BF, "ExternalInput")
    w1o = g("w1o", [128, 128], BF, "ExternalInput")
    b1o = g("b1o_row", [1, 128], BF, "ExternalInput")
    w2o = g("w2op", [128, 128], BF, "ExternalInput")
    b2o = g("b2op_row", [1, 128], BF, "ExternalInput")
    lngr = g("lng_rep", [128, 128], FP, "ExternalInput")
    lnbr = g("lnb_rep", [128, 128], FP, "ExternalInput")
    eps_c = g("eps_col", [128, 1], FP, "ExternalInput")
    b2c = g("b2_col", [128, 1], FP, "ExternalInput")

    out = g("out", [128, NLP], FP, "ExternalOutput")

    SILU = mybir.ActivationFunctionType.Silu
    SIGM = mybir.ActivationFunctionType.Sigmoid
    SQRT = mybir.ActivationFunctionType.Sqrt
    EQ = mybir.AluOpType.is_equal
    MUL = mybir.AluOpType.mult
    ADD = mybir.AluOpType.add
    SUB = mybir.AluOpType.subtract
    BYP = mybir.AluOpType.bypass
    AXX = mybir.AxisListType.X

    with tile.TileContext(nc) as tc:
        with tc.tile_pool(name="const", bufs=1) as cp, \
             tc.tile_pool(name="stream", bufs=1) as sp, \
             tc.tile_pool(name="work", bufs=3) as wk:

            def cload(ap, shape, dt, tag, pool=None):
                t = (pool or cp).tile(shape, dt, tag=tag)
                nc.sync.dma_start(t[:], ap)
                return t

            aloc_s = cload(aloc, [128, NLP], BF, "c_aloc")
            xres_s = cload(xres, [128, NLP], FP, "c_xres")
            selc_s = cload(selc, [128, EP // 128], FP, "c_selc", sp)
            iop_s = cload(iota_p, [128, 512], FP, "c_iop")
            iof_s = cload(iota_f, [128, 128], FP, "c_iof")
            wc_s = cload(wc, [128, 128], BF, "c_wc")
            we_s = cload(we, [1, 128], BF, "c_we")
            w2c_s = cload(w2c, [128, 1], BF, "c_w2c")
            id_s = cload(ident, [128, 128], BF, "c_id")
            ones_s = cload(ones1, [1, 128], BF, "c_ones")
            w1o_s = cload(w1o, [128, 128], BF, "c_w1o")
            b1o_s = cload(b1o, [1, 128], BF, "c_b1o")
            w2o_s = cload(w2o, [128, 128], BF, "c_w2o")
            b2o_s = cload(b2o, [1, 128], BF, "c_b2o")
            lngr_s = cload(lngr, [128, 128], FP, "c_lngr")
            lnbr_s = cload(lnbr, [128, 128], FP, "c_lnbr")
            eps_s = cload(eps_c, [128, 1], FP, "c_eps")
            b2_s = cload(b2c, [128, 1], FP, "c_b2")

            msg_acc = cp.tile([128, NLP], FP, tag="msg_acc")   # msg^T [d, s]

            dma_sems = [nc.alloc_semaphore(f"swdge_dma{q}") for q in range(4)]

            # ---- edge passes -------------------------------------------
            chp = tc.alloc_tile_pool(name="chunk", bufs=2)
            plp = tc.alloc_tile_pool(name="pl", bufs=2, space="PSUM")
            php = tc.alloc_tile_pool(name="ph", bufs=2, space="PSUM")
            pgp = tc.alloc_tile_pool(name="pg", bufs=2, space="PSUM")
            pmp = tc.alloc_tile_pool(name="pm", bufs=2, space="PSUM")
            qn = [0]
            qcnt = [0, 0, 0, 0]

            for pi, (wsizes, chunks, tbase, tab_lo) in enumerate([
                    (WL, chunks_lo, 0, True), (WH, chunks_hi, lo_tiles, False)]):
                if tab_lo:
                    xt_src, xm_src = xtab[0:SPLIT], xmtab[0:SPLIT]
                else:
                    xt_src, xm_src = xtab[SPLIT:N], xmtab[SPLIT:N]
                pass_start = chunks[0][0]

                # window boundaries in pass-tile coordinates
                wbound = []
                j = 0
                for w in range(NW):
                    nt = wsizes[w] // 128
                    wbound.append((j, j + nt))
                    j += nt
                    if nt == 0 and tab_lo:
                        nc.vector.memset(
                            msg_acc[:, w * 128:(w + 1) * 128], 0.0)
                tile_win = np.zeros(j, np.int64)
                for w, (a, b) in enumerate(wbound):
                    tile_win[a:b] = w

                pm_t = [None]

                for ci, (off, gsz) in enumerate(chunks):
                    nct = gsz // 128
                    ct0 = (off - pass_start) // 128    # first pass-tile

                    # -- gathers: prepare_only + trigger on rotating queues
                    idx_c = chp.tile([128, CHUNK // 16], I16, tag="cidx")
                    nc.sync.dma_start(idx_c[:, :gsz // 16],
                                      idxt[:, off // 16:(off + gsz) // 16])
                    xt_c = chp.tile([128, 1, CHUNK], BF, tag="cxT")
                    xm_c = chp.tile([128, CHUNK // 128, 128], BF, tag="cxm")
                    xt_gate = xm_gate = None
                    if PREP_GATHER:
                        q = qn[0] % 4
                        nc.gpsimd.dma_gather(
                            xt_c[:, :, :gsz], xt_src,
                            idx_c[:, :gsz // 16],
                            gsz, gsz, 128, transpose=True,
                            single_packet=False, prepare_only=True,
                            sem=dma_sems[q], queue_num=q)
                        nc.gpsimd.trigger_dma(count=None, queue_num=q)
                        qcnt[q] += 1
                        xt_gate = (dma_sems[q], 16 * qcnt[q])
                        q = (qn[0] + 1) % 4
                        nc.gpsimd.dma_gather(
                            xm_c[:, :gsz // 128, :], xm_src,
                            idx_c[:, :gsz // 16],
                            gsz, gsz, 128, single_packet=False,
                            prepare_only=True, sem=dma_sems[q], queue_num=q)
                        nc.gpsimd.trigger_dma(count=None, queue_num=q)
                        qcnt[q] += 1
                        xm_gate = (dma_sems[q], 16 * qcnt[q])
                        qn[0] += 2
                    else:
                        nc.gpsimd.dma_gather(
                            xt_c[:, :, :gsz], xt_src,
                            idx_c[:, :gsz // 16],
                            gsz, gsz, 128, transpose=True,
                            single_packet=False)
                        nc.gpsimd.dma_gather(
                            xm_c[:, :gsz // 128, :], xm_src,
                            idx_c[:, :gsz // 16],
                            gsz, gsz, 128, single_packet=False)

                    # -- row streams (lrow/ea) in RCH slices
                    rtiles = {}
                    for h in range((gsz + RCH - 1) // RCH):
                        roff = off + h * RCH
                        rsz = min(RCH, off + gsz - roff)
                        lr_c = chp.tile([1, RCH], BF, tag="crow")
                        nc.sync.dma_start(lr_c[:, :rsz],
                                          lrowr[0:1, roff:roff + rsz])
                        ea_c = chp.tile([1, RCH], BF, tag="cea")
                        nc.sync.dma_start(ea_c[:, :rsz],
                                          ear[0:1, roff:roff + rsz])
                        rtiles[h] = (lr_c, ea_c)

                    # -- compute pass: blocks of <= BLKT tiles
                    pg = pgp.tile([128, CHUNK // 128], FP, tag="pgl")
                    jt = ct0
                    while jt < ct0 + nct:
                        w = int(tile_win[jt])
                        h = (jt - ct0) // (RCH // 128)
                        jend = min(jt + BLKT, wbound[w][1], ct0 + nct,
                                   ct0 + (h + 1) * (RCH // 128))
                        bw = (jend - jt) * 128
                        e0 = (jt - ct0) * 128             # chunk-local
                        eR = (jt - ct0) * 128 - h * RCH   # rch-local
                        lr_c, ea_c = rtiles[h]

                        prep = plp.tile([128, 512], FP, tag="plrep")
                        nc.tensor.matmul(prep[:, :bw], ones_s[:],
                                         lr_c[:, eR:eR + bw],
                                         start=True, stop=True)
                        oht = wk.tile([128, 512], BF, tag="oht")
                        nc.vector.tensor_tensor(oht[:, :bw], iop_s[:, :bw],
                                                prep[:, :bw], EQ)
                        ph_t = php.tile([128, 512], FP, tag="phid")
                        nc.tensor.matmul(ph_t[:, :bw],
                                         aloc_s[:, w * 128:(w + 1) * 128],
                                         oht[:, :bw], start=True, stop=False)
                        mi = nc.tensor.matmul(ph_t[:, :bw], wc_s[:],
                                              xt_c[:, 0, e0:e0 + bw],
                                              start=False, stop=False)
                        if xt_gate is not None:
                            mi.wait_op(xt_gate[0], xt_gate[1], "sem-ge")
                        nc.tensor.matmul(ph_t[:, :bw], we_s[:],
                                         ea_c[:, eR:eR + bw],
                                         start=False, stop=True)
                        silT = wk.tile([128, 512], BF, tag="silT")
                        nc.scalar.activation(silT[:, :bw], ph_t[:, :bw], SILU)
                        for t in range(jt, jend):
                            ctl = t - ct0
                            nc.tensor.matmul(
                                pg[:, ctl:ctl + 1],
                                silT[:, (t - jt) * 128:(t - jt + 1) * 128],
                                w2c_s[:], start=True, stop=True)
                        jt = jend

                    # -- batched sigmoid for the whole chunk
                    attn = wk.tile([128, CHUNK // 128, ], FP, tag="attn")
                    nc.scalar.activation(attn[:, :nct], pg[:, :nct], SIGM,
                                         bias=b2_s[:, 0:1])

                    # -- scatter pass
                    for t in range(ct0, ct0 + nct):
                        w = int(tile_win[t])
                        ws, wend = wbound[w]
                        ctl = t - ct0
                        gt = tbase + t
                        if t == ws:
                            pm_t[0] = pmp.tile([128, 128], FP, tag="pmsg",
                                               name="pmsg")
                        sel = wk.tile([128, 128], BF, tag="sel")
                        nc.vector.tensor_scalar(sel[:], iof_s[:],
                                                selc_s[:, gt:gt + 1],
                                                attn[:, ctl:ctl + 1], EQ, MUL)
                        mi = nc.tensor.matmul(pm_t[0][:], xm_c[:, ctl, :],
                                              sel[:], start=(t == ws),
                                              stop=(t == wend - 1))
                        if xm_gate is not None:
                            mi.wait_op(xm_gate[0], xm_gate[1], "sem-ge")
                        if t == wend - 1:
                            wblk = msg_acc[:, w * 128:(w + 1) * 128]
                            if tab_lo:
                                nc.vector.tensor_copy(wblk, pm_t[0][:])
                            else:
                                nc.vector.tensor_tensor(
                                    wblk, pm_t[0][:], wblk, ADD)

            for _p in (pmp, pgp, php, plp, chp):
                _p.release()

            # ---- out stage (two-pass, batched LN stats) ----------------
            ptp = tc.alloc_tile_pool(name="pt", bufs=2, space="PSUM")
            pop = tc.alloc_tile_pool(name="po", bufs=2, space="PSUM")
            s1buf = cp.tile([128, NLP], FP, tag="o_s1buf")
            musum = cp.tile([128, NW], FP, tag="o_musum")
            sqsum = cp.tile([128, NW], FP, tag="o_sqsum")
            mu2s = cp.tile([128, NW], FP, tag="o_mu2s")
            sq2s = cp.tile([128, NW], FP, tag="o_sq2s")

            for w in range(NW):
                wblk = slice(w * 128, (w + 1) * 128)
                mbf = wk.tile([128, 128], BF, tag="o_mbf")
                nc.vector.tensor_copy(mbf[:], msg_acc[:, wblk])
                po1 = pop.tile([128, 128], FP, tag="o_po1")
                nc.tensor.matmul(po1[:], mbf[:], w1o_s[:], start=True,
                                 stop=False)
                nc.tensor.matmul(po1[:], ones_s[:], b1o_s[:], start=False,
                                 stop=True)
                nc.scalar.activation(s1buf[:, wblk], po1[:], SILU,
                                     accum_out=musum[:, w:w + 1])
                jk = wk.tile([128, 128], BF, tag="o_jk")
                nc.vector.scalar_tensor_tensor(
                    jk[:], s1buf[:, wblk], 0.0, s1buf[:, wblk], BYP, MUL,
                    accum_out=sqsum[:, w:w + 1])

            def batch_stats(msum, sqs, tagp):
                mu = cp.tile([128, NW], FP, tag=f"{tagp}_mu")
                nc.vector.tensor_scalar(mu[:], msum[:], 1.0 / 128.0, None, MUL)
                musq = wk.tile([128, NW], FP, tag=f"{tagp}_musq")
                nc.vector.tensor_tensor(musq[:], mu[:], mu[:], MUL)
                var = wk.tile([128, NW], FP, tag=f"{tagp}_var")
                nc.vector.tensor_scalar(var[:], sqs[:], 1.0 / 128.0, None, MUL)
                var2 = wk.tile([128, NW], FP, tag=f"{tagp}_var2")
                nc.vector.tensor_tensor(var2[:], var[:], musq[:], SUB)
                std = wk.tile([128, NW], FP, tag=f"{tagp}_std")
                nc.scalar.activation(std[:], var2[:], SQRT, bias=eps_s[:, 0:1])
                rstd = cp.tile([128, NW], FP, tag=f"{tagp}_rstd")
                nc.vector.reciprocal(rstd[:], std[:])
                return mu, rstd

            mu1, rstd1 = batch_stats(musum, sqsum, "bs1")

            for w in range(NW):
                wblk = slice(w * 128, (w + 1) * 128)
                z = wk.tile([128, 128], BF, tag="o_z")
                nc.vector.tensor_scalar(z[:], s1buf[:, wblk], mu1[:, w:w + 1],
                                        rstd1[:, w:w + 1], SUB, MUL)
                pzt = ptp.tile([128, 128], BF, tag="o_pzt")
                nc.tensor.transpose(pzt[:], z[:], id_s[:])
                zt = wk.tile([128, 128], BF, tag="o_zt")
                nc.vector.tensor_copy(zt[:], pzt[:])
                po2 = pop.tile([128, 128], FP, tag="o_po2")
                nc.tensor.matmul(po2[:], zt[:], w2o_s[:], start=True,
                                 stop=False)
                nc.tensor.matmul(po2[:], ones_s[:], b2o_s[:], start=False,
                                 stop=True)
                # r = x + out_mlp(msg); overwrite s1buf window
                nc.vector.tensor_tensor(s1buf[:, wblk], po2[:],
                                        xres_s[:, wblk], ADD)
                nc.vector.reduce_sum(mu2s[:, w:w + 1], s1buf[:, wblk],
                                     axis=AXX)
                jk2 = wk.tile([128, 128], BF, tag="o_jk2")
                nc.vector.scalar_tensor_tensor(
                    jk2[:], s1buf[:, wblk], 0.0, s1buf[:, wblk], BYP, MUL,
                    accum_out=sq2s[:, w:w + 1])

            mu2, rstd2 = batch_stats(mu2s, sq2s, "bs2")

            for w in range(NW):
                wblk = slice(w * 128, (w + 1) * 128)
                zf = wk.tile([128, 128], FP, tag="o_zf")
                nc.vector.tensor_scalar(zf[:], s1buf[:, wblk],
                                        mu2[:, w:w + 1], rstd2[:, w:w + 1],
                                        SUB, MUL)
                zg = wk.tile([128, 128], FP, tag="o_zg")
                nc.vector.tensor_tensor(zg[:], zf[:], lngr_s[:], MUL)
                ot = wk.tile([128, 128], FP, tag="o_ot")
                nc.vector.tensor_tensor(ot[:], zg[:], lnbr_s[:], ADD)
                nc.sync.dma_start(out[:, wblk], ot[:])
            pop.release()
            ptp.release()

    nc.compile()
    return nc


def _phase2_inputs(plan, streams, p1_results, inp, inp2_b2):
    """Assemble per-core phase-2 in_maps from phase-1 outputs."""
    att_w1 = inp["att_w1"]
    att_w2 = inp["att_w2"]

    def unwrap(a, dt=F32):
        a = np.asarray(a, dtype=dt) if dt is not None else np.asarray(a)
        return a.reshape(128, NW, 128).transpose(1, 0, 2).reshape(NLP, 128)

    # global tables (bf16), padded to N rows only
    xtab = np.concatenate(
        [unwrap(p1_results[c]["x_out"])[:NL] for c in range(NCORES)])
    xmtab = np.concatenate(
        [unwrap(np.asarray(p1_results[c]["xm_out"], F32))[:NL]
         for c in range(NCORES)])
    xtab = _bf(xtab)
    xmtab = _bf(xmtab)

    w2op = inp["out_ln_g"][:, None] * inp["out_w2"]
    b2op = inp["out_ln_b"] @ inp["out_w2"] + inp["out_b2"]
    consts = {
        "xtab": xtab, "xmtab": xmtab,
        "iota_part": _f32(np.broadcast_to(np.arange(128)[:, None], (128, 512))),
        "iota_free": _f32(np.broadcast_to(np.arange(128)[None, :], (128, 128))),
        "wc": _bf(att_w1[D:2 * D]), "we_row": _bf(att_w1[2 * D:2 * D + 1]),
        "w2col": _bf(att_w2[:, 0:1]),
        "ident": _bf(np.eye(128)), "ones1": _bf(np.ones((1, 128))),
        "w1o": _bf(inp["out_w1"]), "b1o_row": _bf(inp["out_b1"][None, :]),
        "w2op": _bf(w2op), "b2op_row": _bf(b2op[None, :]),
        "lng_rep": _f32(np.broadcast_to(inp["ln_g"][None, :], (128, 128))),
        "lnb_rep": _f32(np.broadcast_to(inp["ln_b"][None, :], (128, 128))),
        "eps_col": _f32(np.full((128, 1), LN_EPS)),
        "b2_col": _f32(np.full((128, 1), inp2_b2)),
    }
    in_maps = []
    for c in range(NCORES):
        in_maps.append({
            "aloc": np.ascontiguousarray(np.asarray(p1_results[c]["a_out"])),
            "xres": np.ascontiguousarray(np.asarray(p1_results[c]["x_out"])),
            "sel_col": streams["sel_col"][c],
            "idx": streams["idx"][c],
            "lrow_row": streams["lrow_row"][c],
            "ea_row": streams["ea_row"][c],
            **consts,
        })
    return in_maps


def kernel(**inputs):
    inp = {k: np.asarray(v) for k, v in inputs.items()}
    h = _f32(inp["h"])
    weights = (inp["lin_w"], inp["lin_b"], inp["msg_w1"], inp["msg_b1"],
               inp["msg_ln_g"], inp["msg_ln_b"], inp["msg_w2"], inp["msg_b2"],
               inp["att_w1"], inp["att_b1"])
    b2 = float(np.asarray(inp["att_b2"]).ravel()[0])
    plan, streams = _prep_edges(inp["row"], inp["col"], inp["edge_attr"],
                                inp["edge_mask"])

    p1 = _run_phase1(h, weights)

    key = (tuple(plan["WL"]), tuple(plan["WH"]))
    if _cache.get("p2_key") != key:
        _cache["p2"] = _build_phase2(plan)
        _cache["p2_key"] = key
    nc2 = _cache["p2"]
    in_maps = _phase2_inputs(plan, streams, p1, inp, b2)
    res = run_bass_kernel_spmd(nc2, in_maps, core_ids=list(range(NCORES)),
                               trace=TRACE)
    if TRACE:
        LAST_RESULTS["phase2_ns"] = res.exec_time_ns
    out = np.concatenate([
        np.asarray(res.results[c]["out"], F32)
        .reshape(128, NW, 128).transpose(1, 0, 2).reshape(NLP, 128)[:NL]
        for c in range(NCORES)])
    return out.astype(F32)


# revision 23
# speedup vs baseline: 1.0006x; 1.0006x over previous
"""GCLayer (GNN message passing) on 8 Trainium2 NeuronCores.

Strategy
--------
Edges are partitioned by destination row (node-range sharding): core c owns
nodes [c*6250, (c+1)*6250) and every edge whose `row` lands there, so the
segment-sum needs no cross-core reduction.

Phase 1 (node stage, sharded): x = h@lin_w+b, x_ = msg_mlp(x),
A = x@att_w1[:D]+att_b1 per local node (row-major).  The host assembles the
global x / x_ tables (bf16) and feeds them to every core.

Phase 2 (edge + out stage), feature-major ("flipped") attention:
  hid^T[j, e] = A^T-gather (aloc lhsT @ one-hot rhs) + wc lhsT @ x^T[col]
                + we outer ea           (PSUM, blocks up to 512 edges wide)
  sil^T = SILU(hid^T)                   (ACT, one op per block)
  logit[e] = sil^T slice (lhsT) @ w2col (PE does the w2 dot, one col/tile)
  att = sigmoid(logits + b2)            (ACT, ONE op per 64-tile chunk ->
                                         no per-tile SILU/SIGMOID table churn)
  sel[e, s] = att_e * (iota_f == lrow_e) (vector tensor_scalar, fused)
  msg^T[d, s] += xm[col] (lhsT) @ sel   (PE scatter, transposed msg)
Masking/padding is exact via a sentinel lrow (=200) so sel columns of pad or
masked edges are all-zero; no -30000 bias hack needed.

x^T[col] / xm[col] are fetched with SWDGE dma_gather in prepare_only mode +
trigger_dma on 4 SWDGE queues, so the gpsimd engine only pays descriptor
generation and the random-access HBM transfers overlap compute.

LayerNorms (phase 1, out stage) are two-pass with batched stats: SILU keeps
its ACT table resident; Var = E[x^2]-mu^2 via scalar_tensor_tensor accum;
sqrt over all 49 windows in one ACT op.
"""

import sys

sys.path.insert(0, "/opt/trn_rl_repo")

import numpy as np
import ml_dtypes

from concourse import bacc, mybir, tile
from concourse.bass_utils import run_bass_kernel_spmd

BF16 = ml_dtypes.bfloat16
F32 = np.float32

NCORES = 8
N = 50000
E = 800000
D = 128
NL = N // NCORES          # 6250 real nodes per core
NW = 49                   # node blocks of 128 per core (49*128 = 6272)
NLP = NW * 128            # padded nodes per core
SPLIT = 32768             # int16 gather index limit
CHUNK = 8192              # edges per dma_gather call (64 tiles)
RCH = 2048                # row-stream load granularity (16 tiles)
BLKT = 4                  # tiles per hidden block (<=512 edges)
PREP_GATHER = True        # prepare_only + trigger_dma (overlapped transfers)
LN_EPS = 1e-5
SENT = 200.0              # sentinel lrow for pad/masked edges (> 127)

FP = mybir.dt.float32
BF = mybir.dt.bfloat16
I16 = mybir.dt.int16

TRACE = False             # test.py sets kernel.TRACE = True for profiling
LAST_RESULTS = {}         # exec_time_ns per phase when TRACE

_cache = {}


def _bf(a):
    return np.ascontiguousarray(np.asarray(a, dtype=F32).astype(BF16))


def _f32(a):
    return np.ascontiguousarray(np.asarray(a, dtype=F32))


def _ceil(a, m):
    return -(-int(a) // m) * m


# ---------------------------------------------------------------------------
# Host-side edge preprocessing
# ---------------------------------------------------------------------------

def _prep_edges(row, col, ea, em):
    """Sort/partition/pad edges. Returns per-core streams + the static plan."""
    row = np.asarray(row).astype(np.int64).ravel()
    col = np.asarray(col).astype(np.int64).ravel()
    ea = np.asarray(ea, dtype=F32).ravel()
    em = np.asarray(em, dtype=F32).ravel()

    c_of = row // NL
    r_loc = row - c_of * NL
    w_of = r_loc // 128
    s_in_w = r_loc % 128
    hi = (col >= SPLIT).astype(np.int64)

    # bucket = (core, pass(lo/hi), window); argsort gives the stream order
    key = (c_of * 2 + hi) * NW + w_of
    order = np.argsort(key, kind="stable")
    skey = key[order]
    cnt = np.bincount(key, minlength=NCORES * 2 * NW).reshape(NCORES, 2, NW)

    WL = np.array([_ceil(cnt[:, 0, w].max(), 128) for w in range(NW)])
    WH = np.array([_ceil(cnt[:, 1, w].max(), 128) for w in range(NW)])
    lo_total = int(WL.sum())
    hi_total = int(WH.sum())
    EP = lo_total + hi_total

    # padded base offset of each (pass, window) block within the stream
    sizes = np.concatenate([WL, WH])                      # (2*NW,)
    base = np.concatenate([[0], np.cumsum(sizes)[:-1]])   # (2*NW,)

    # rank of each edge within its bucket
    bstart = np.concatenate([[0], np.cumsum(cnt.ravel())[:-1]])
    rank = np.arange(row.size) - bstart[skey]
    pw = skey % (2 * NW)                                   # (pass, window) id
    dest = base[pw] + rank                                 # position in stream
    cc = skey // (2 * NW)

    g_lrow = np.zeros((NCORES, EP), F32)
    g_sel = np.full((NCORES, EP), SENT, F32)   # sentinel: pad edges -> sel 0
    g_idx = np.zeros((NCORES, EP), np.int16)
    g_ea = np.zeros((NCORES, EP), F32)

    e_ids = order
    g_lrow[cc, dest] = s_in_w[e_ids].astype(F32)
    # masked real edges keep the sentinel too (exact edge_mask handling)
    lsel = np.where(em[e_ids] > 0.5, s_in_w[e_ids].astype(F32), SENT)
    g_sel[cc, dest] = lsel
    g_idx[cc, dest] = (col[e_ids] - hi[e_ids] * SPLIT).astype(np.int16)
    g_ea[cc, dest] = ea[e_ids]

    # wrapped layouts
    sel_col = np.ascontiguousarray(
        g_sel.reshape(NCORES, EP // 128, 128).transpose(0, 2, 1))
    idx16 = g_idx.reshape(NCORES, EP // 16, 16).transpose(0, 2, 1)  # (8,16,EP/16)
    idx_w = np.ascontiguousarray(np.tile(idx16, (1, 8, 1)))         # (8,128,EP/16)
    lrow_row = _bf(g_lrow.reshape(NCORES, 1, EP))
    ea_row = _bf(g_ea.reshape(NCORES, 1, EP))

    def chunks(total, start):
        out = []
        off = 0
        while off < total:
            g = min(CHUNK, total - off)
            out.append((start + off, g))
            off += g
        return out

    plan = dict(
        WL=[int(x) for x in WL], WH=[int(x) for x in WH], EP=EP,
        lo_total=lo_total, hi_total=hi_total,
        chunks_lo=chunks(lo_total, 0), chunks_hi=chunks(hi_total, lo_total),
    )
    streams = dict(sel_col=sel_col, idx=idx_w,
                   lrow_row=lrow_row, ea_row=ea_row)
    return plan, streams


# ---------------------------------------------------------------------------
# Phase 1: node stage (sharded over nodes)
# ---------------------------------------------------------------------------

def _build_phase1():
    nc = bacc.Bacc("TRN2", target_bir_lowering=False, debug=False,
                   num_devices=NCORES)
    g = lambda n, s, d, k: nc.dram_tensor(n, s, d, kind=k).ap()

    ht = g("ht", [128, NLP], BF, "ExternalInput")          # h^T, node-wrapped
    linw = g("linw", [128, 128], BF, "ExternalInput")
    linb = g("linb_row", [1, 128], BF, "ExternalInput")
    w1m = g("w1m", [128, 128], BF, "ExternalInput")
    b1m = g("b1m_row", [1, 128], BF, "ExternalInput")
    w2mp = g("w2mp", [128, 128], BF, "ExternalInput")
    b2mp = g("b2mp_row", [1, 128], BF, "ExternalInput")
    wr = g("wr", [128, 128], BF, "ExternalInput")
    b1a = g("b1a_row", [1, 128], BF, "ExternalInput")
    ident = g("ident", [128, 128], BF, "ExternalInput")
    ones1 = g("ones1", [1, 128], BF, "ExternalInput")

    eps_c = g("eps_col", [128, 1], FP, "ExternalInput")

    x_out = g("x_out", [128, NLP], FP, "ExternalOutput")
    xm_out = g("xm_out", [128, NLP], BF, "ExternalOutput")
    a_out = g("a_out", [128, NLP], BF, "ExternalOutput")

    SILU = mybir.ActivationFunctionType.Silu
    SQRT = mybir.ActivationFunctionType.Sqrt
    MUL = mybir.AluOpType.mult
    SUB = mybir.AluOpType.subtract
    BYP = mybir.AluOpType.bypass

    with tile.TileContext(nc) as tc:
        with tc.tile_pool(name="const", bufs=1) as cp, \
             tc.tile_pool(name="work", bufs=3) as wp, \
             tc.tile_pool(name="psum", bufs=1, space="PSUM") as pp, \
             tc.tile_pool(name="psum2", bufs=2, space="PSUM") as pp2:

            def cload(ap, shape, dt, tag):
                t = cp.tile(shape, dt, tag=tag)
                nc.sync.dma_start(t[:], ap)
                return t

            ht_s = cload(ht, [128, NLP], BF, "c_ht")
            linw_s = cload(linw, [128, 128], BF, "c_linw")
            linb_s = cload(linb, [1, 128], BF, "c_linb")
            w1m_s = cload(w1m, [128, 128], BF, "c_w1m")
            b1m_s = cload(b1m, [1, 128], BF, "c_b1m")
            w2mp_s = cload(w2mp, [128, 128], BF, "c_w2mp")
            b2mp_s = cload(b2mp, [1, 128], BF, "c_b2mp")
            wr_s = cload(wr, [128, 128], BF, "c_wr")
            b1a_s = cload(b1a, [1, 128], BF, "c_b1a")
            id_s = cload(ident, [128, 128], BF, "c_id")
            ones_s = cload(ones1, [1, 128], BF, "c_ones")
            eps_s = cload(eps_c, [128, 1], FP, "c_eps")

            s1buf = cp.tile([128, NLP], FP, tag="s1buf")
            musum = cp.tile([128, NW], FP, tag="musum")
            sqsum = cp.tile([128, NW], FP, tag="sqsum")
            xtbuf = cp.tile([128, NLP], BF, tag="xtbuf")

            # ---- pass A: x, x^T, A, s1 = silu(mlp1(x)), LN stats ---------
            for w in range(NW):
                blk = slice(w * 128, (w + 1) * 128)
                htb = ht_s[:, blk]

                # x = h @ lin_w + lin_b      [n, d]  (row-major)
                px = pp.tile([128, 128], FP, tag="px")
                nc.tensor.matmul(px[:], htb, linw_s[:], start=True, stop=False)
                nc.tensor.matmul(px[:], ones_s[:], linb_s[:], start=False,
                                 stop=True)
                xs = wp.tile([128, 128], FP, tag="xs")
                nc.scalar.copy(xs[:], px[:])
                nc.sync.dma_start(x_out[:, blk], xs[:])

                # x^T  (for downstream lhsT use)
                pxt = pp.tile([128, 128], FP, tag="pxt")
                nc.tensor.matmul(pxt[:], linw_s[:], htb, start=True, stop=False)
                nc.tensor.matmul(pxt[:], linb_s[:], ones_s[:], start=False,
                                 stop=True)
                nc.vector.tensor_copy(xtbuf[:, blk], pxt[:])

                # A = x @ wr + b1a           [n, h]  (row-major)
                pa = pp.tile([128, 128], FP, tag="pa")
                nc.tensor.matmul(pa[:], xtbuf[:, blk], wr_s[:], start=True,
                                 stop=False)
                nc.tensor.matmul(pa[:], ones_s[:], b1a_s[:], start=False,
                                 stop=True)
                asb = wp.tile([128, 128], BF, tag="asb")
                nc.scalar.copy(asb[:], pa[:])
                nc.sync.dma_start(a_out[:, blk], asb[:])

                # s1 = silu(x @ w1m + b1m), musum = sum(s1)
                ps = pp.tile([128, 128], FP, tag="ps")
                nc.tensor.matmul(ps[:], xtbuf[:, blk], w1m_s[:], start=True,
                                 stop=False)
                nc.tensor.matmul(ps[:], ones_s[:], b1m_s[:], start=False,
                                 stop=True)
                nc.scalar.activation(s1buf[:, blk], ps[:], SILU,
                                     accum_out=musum[:, w:w + 1])
                jk = wp.tile([128, 128], BF, tag="jk")
                nc.vector.scalar_tensor_tensor(
                    jk[:], s1buf[:, blk], 0.0, s1buf[:, blk], BYP, MUL,
                    accum_out=sqsum[:, w:w + 1])

            # ---- batched LN stats ---------------------------------------
            mu = cp.tile([128, NW], FP, tag="mu")
            nc.vector.tensor_scalar(mu[:], musum[:], 1.0 / 128.0, None, MUL)
            musq = wp.tile([128, NW], FP, tag="musq")
            nc.vector.tensor_tensor(musq[:], mu[:], mu[:], MUL)
            var = wp.tile([128, NW], FP, tag="var")
            nc.vector.tensor_scalar(var[:], sqsum[:], 1.0 / 128.0, None, MUL)
            var2 = wp.tile([128, NW], FP, tag="var2")
            nc.vector.tensor_tensor(var2[:], var[:], musq[:], SUB)
            std = wp.tile([128, NW], FP, tag="std")
            nc.scalar.activation(std[:], var2[:], SQRT, bias=eps_s[:, 0:1])
            rstd = cp.tile([128, NW], FP, tag="rstd")
            nc.vector.reciprocal(rstd[:], std[:])

            # ---- pass B: z -> x_ = z @ w2mp + b2mp ----------------------
            for w in range(NW):
                blk = slice(w * 128, (w + 1) * 128)
                z = wp.tile([128, 128], BF, tag="z")
                nc.vector.tensor_scalar(z[:], s1buf[:, blk], mu[:, w:w + 1],
                                        rstd[:, w:w + 1], SUB, MUL)
                pzt = pp2.tile([128, 128], BF, tag="pzt")
                nc.tensor.transpose(pzt[:], z[:], id_s[:])
                zt = wp.tile([128, 128], BF, tag="zt")
                nc.vector.tensor_copy(zt[:], pzt[:])
                pxm = pp.tile([128, 128], FP, tag="pxm")
                nc.tensor.matmul(pxm[:], zt[:], w2mp_s[:], start=True,
                                 stop=False)
                nc.tensor.matmul(pxm[:], ones_s[:], b2mp_s[:], start=False,
                                 stop=True)
                xm = wp.tile([128, 128], BF, tag="xm")
                nc.scalar.copy(xm[:], pxm[:])
                nc.sync.dma_start(xm_out[:, blk], xm[:])

    nc.compile()
    return nc


def _phase1_inputs(h, weights):
    """Per-core in_maps for phase 1."""
    (lin_w, lin_b, msg_w1, msg_b1, msg_ln_g, msg_ln_b, msg_w2, msg_b2,
     att_w1, att_b1) = weights
    w2mp = msg_ln_g[:, None] * msg_w2
    b2mp = msg_ln_b @ msg_w2 + msg_b2
    consts = {
        "linw": _bf(lin_w), "linb_row": _bf(lin_b[None, :]),
        "w1m": _bf(msg_w1), "b1m_row": _bf(msg_b1[None, :]),
        "w2mp": _bf(w2mp), "b2mp_row": _bf(b2mp[None, :]),
        "wr": _bf(att_w1[0:D]), "b1a_row": _bf(att_b1[None, :]),
        "ident": _bf(np.eye(128)), "ones1": _bf(np.ones((1, 128))),
        "eps_col": _f32(np.full((128, 1), LN_EPS)),
    }
    in_maps = []
    for c in range(NCORES):
        hc = np.zeros((NLP, D), F32)
        hc[:NL] = h[c * NL:(c + 1) * NL]
        in_maps.append({"ht": _bf(hc.T), **consts})
    return in_maps


def _run_phase1(h, weights):
    if "p1" not in _cache:
        _cache["p1"] = _build_phase1()
    nc = _cache["p1"]
    in_maps = _phase1_inputs(h, weights)
    res = run_bass_kernel_spmd(nc, in_maps, core_ids=list(range(NCORES)),
                               trace=TRACE)
    if TRACE:
        LAST_RESULTS["phase1_ns"] = res.exec_time_ns
    return res.results


# ---------------------------------------------------------------------------
# Phase 2: edge stage (gather/attention/scatter) + out stage
# ---------------------------------------------------------------------------

def _build_phase2(plan):
    WL, WH, EP = plan["WL"], plan["WH"], plan["EP"]
    chunks_lo, chunks_hi = plan["chunks_lo"], plan["chunks_hi"]
    lo_tiles = plan["lo_total"] // 128

    nc = bacc.Bacc("TRN2", target_bir_lowering=False, debug=False,
                   num_devices=NCORES, num_swdge_queues=4)
    g = lambda n, s, d, k: nc.dram_tensor(n, s, d, kind=k).ap()

    xtab = g("xtab", [N, 128], BF, "ExternalInput")
    xmtab = g("xmtab", [N, 128], BF, "ExternalInput")
    aloc = g("aloc", [128, NLP], BF, "ExternalInput")      # A rows per window
    xres = g("xres", [128, NLP], FP, "ExternalInput")      # x rows per window
    selc = g("sel_col", [128, EP // 128], FP, "ExternalInput")
    idxt = g("idx", [128, EP // 16], I16, "ExternalInput")
    lrowr = g("lrow_row", [1, EP], BF, "ExternalInput")
    ear = g("ea_row", [1, EP], BF, "ExternalInput")

    iota_p = g("iota_part", [128, 512], FP, "ExternalInput")
    iota_f = g("iota_free", [128, 128], FP, "ExternalInput")
    wc = g("wc", [128, 128], BF, "ExternalInput")
    we = g("we_row", [1, 128], BF, "ExternalInput")
    w2c = g("w2col", [128, 1], BF, "ExternalInput")
    ident = g("ident", [128, 128], BF, "ExternalInput")
    ones1 = g("ones1", [1, 128], BF, "ExternalInput")
    w1o = g("w1o", [128, 128], BF, "ExternalInput")
    b1o = g("b1o_row", [1, 128], BF, "ExternalInput")
    w2o = g("w2op", [128, 128], BF, "ExternalInput")
    b2o = g("b2op_row", [1, 128], BF, "ExternalInput")
    lngr = g("lng_rep", [128, 128], FP, "ExternalInput")
    lnbr = g("lnb_rep", [128, 128], FP, "ExternalInput")
    eps_c = g("eps_col", [128, 1], FP, "ExternalInput")
    b2c = g("b2_col", [128, 1], FP, "ExternalInput")

    out = g("out", [128, NLP], FP, "ExternalOutput")

    SILU = mybir.ActivationFunctionType.Silu
    SIGM = mybir.ActivationFunctionType.Sigmoid
    SQRT = mybir.ActivationFunctionType.Sqrt
    EQ = mybir.AluOpType.is_equal
    MUL = mybir.AluOpType.mult
    ADD = mybir.AluOpType.add
    SUB = mybir.AluOpType.subtract
    BYP = mybir.AluOpType.bypass
    AXX = mybir.AxisListType.X

    with tile.TileContext(nc) as tc:
        with tc.tile_pool(name="const", bufs=1) as cp, \
             tc.tile_pool(name="stream", bufs=1) as sp, \
             tc.tile_pool(name="work", bufs=3) as wk:

            def cload(ap, shape, dt, tag, pool=None):
                t = (pool or cp).tile(shape, dt, tag=tag)
                nc.sync.dma_start(t[:], ap)
                return t

            aloc_s = cload(aloc, [128, NLP], BF, "c_aloc")
            xres_s = cload(xres, [128, NLP], FP, "c_xres")
            selc_s = cload(selc, [128, EP // 128], FP, "c_selc", sp)
            iop_s = cload(iota_p, [128, 512], FP, "c_iop")
            iof_s = cload(iota_f, [128, 128], FP, "c_iof")
            wc_s = cload(wc, [128, 128], BF, "c_wc")
            we_s = cload(we, [1, 128], BF, "c_we")
            w2c_s = cload(w2c, [128, 1], BF, "c_w2c")
            id_s = cload(ident, [128, 128], BF, "c_id")
            ones_s = cload(ones1, [1, 128], BF, "c_ones")
            w1o_s = cload(w1o, [128, 128], BF, "c_w1o")
            b1o_s = cload(b1o, [1, 128], BF, "c_b1o")
            w2o_s = cload(w2o, [128, 128], BF, "c_w2o")
            b2o_s = cload(b2o, [1, 128], BF, "c_b2o")
            lngr_s = cload(lngr, [128, 128], FP, "c_lngr")
            lnbr_s = cload(lnbr, [128, 128], FP, "c_lnbr")
            eps_s = cload(eps_c, [128, 1], FP, "c_eps")
            b2_s = cload(b2c, [128, 1], FP, "c_b2")

            msg_acc = cp.tile([128, NLP], FP, tag="msg_acc")   # msg^T [d, s]

            dma_sems = [nc.alloc_semaphore(f"swdge_dma{q}") for q in range(4)]

            # ---- edge passes -------------------------------------------
            chp = tc.alloc_tile_pool(name="chunk", bufs=2)
            plp = tc.alloc_tile_pool(name="pl", bufs=2, space="PSUM")
            php = tc.alloc_tile_pool(name="ph", bufs=2, space="PSUM")
            pgp = tc.alloc_tile_pool(name="pg", bufs=2, space="PSUM")
            pmp = tc.alloc_tile_pool(name="pm", bufs=2, space="PSUM")
            qn = [0]
            qcnt = [0, 0, 0, 0]

            for pi, (wsizes, chunks, tbase, tab_lo) in enumerate([
                    (WL, chunks_lo, 0, True), (WH, chunks_hi, lo_tiles, False)]):
                if tab_lo:
                    xt_src, xm_src = xtab[0:SPLIT], xmtab[0:SPLIT]
                else:
                    xt_src, xm_src = xtab[SPLIT:N], xmtab[SPLIT:N]
                pass_start = chunks[0][0]

                # window boundaries in pass-tile coordinates
                wbound = []
                j = 0
                for w in range(NW):
                    nt = wsizes[w] // 128
                    wbound.append((j, j + nt))
                    j += nt
                    if nt == 0 and tab_lo:
                        nc.vector.memset(
                            msg_acc[:, w * 128:(w + 1) * 128], 0.0)
                tile_win = np.zeros(j, np.int64)
                for w, (a, b) in enumerate(wbound):
                    tile_win[a:b] = w

                pm_t = [None]

                for ci, (off, gsz) in enumerate(chunks):
                    nct = gsz // 128
                    ct0 = (off - pass_start) // 128    # first pass-tile

                    # -- gathers: prepare_only + trigger on rotating queues
                    idx_c = chp.tile([128, CHUNK // 16], I16, tag="cidx")
                    nc.sync.dma_start(idx_c[:, :gsz // 16],
                                      idxt[:, off // 16:(off + gsz) // 16])
                    xt_c = chp.tile([128, 1, CHUNK], BF, tag="cxT")
                    xm_c = chp.tile([128, CHUNK // 128, 128], BF, tag="cxm")
                    xt_gate = xm_gate = None
                    if PREP_GATHER:
                        q = qn[0] % 4
                        nc.gpsimd.dma_gather(
                            xt_c[:, :, :gsz], xt_src,
                            idx_c[:, :gsz // 16],
                            gsz, gsz, 128, transpose=True,
                            single_packet=False, prepare_only=True,
                            sem=dma_sems[q], queue_num=q)
                        nc.gpsimd.trigger_dma(count=None, queue_num=q)
                        qcnt[q] += 1
                        xt_gate = (dma_sems[q], 16 * qcnt[q])
                        q = (qn[0] + 1) % 4
                        nc.gpsimd.dma_gather(
                            xm_c[:, :gsz // 128, :], xm_src,
                            idx_c[:, :gsz // 16],
                            gsz, gsz, 128, single_packet=False,
                            prepare_only=True, sem=dma_sems[q], queue_num=q)
                        nc.gpsimd.trigger_dma(count=None, queue_num=q)
                        qcnt[q] += 1
                        xm_gate = (dma_sems[q], 16 * qcnt[q])
                        qn[0] += 2
                    else:
                        nc.gpsimd.dma_gather(
                            xt_c[:, :, :gsz], xt_src,
                            idx_c[:, :gsz // 16],
                            gsz, gsz, 128, transpose=True,
                            single_packet=False)
                        nc.gpsimd.dma_gather(
                            xm_c[:, :gsz // 128, :], xm_src,
                            idx_c[:, :gsz // 16],
                            gsz, gsz, 128, single_packet=False)

                    # -- row streams (lrow/ea) in RCH slices
                    rtiles = {}
                    for h in range((gsz + RCH - 1) // RCH):
                        roff = off + h * RCH
                        rsz = min(RCH, off + gsz - roff)
                        lr_c = chp.tile([1, RCH], BF, tag="crow")
                        nc.sync.dma_start(lr_c[:, :rsz],
                                          lrowr[0:1, roff:roff + rsz])
                        ea_c = chp.tile([1, RCH], BF, tag="cea")
                        nc.sync.dma_start(ea_c[:, :rsz],
                                          ear[0:1, roff:roff + rsz])
                        rtiles[h] = (lr_c, ea_c)

                    # -- compute pass: blocks of <= BLKT tiles
                    pg = pgp.tile([128, CHUNK // 128], FP, tag="pgl")
                    jt = ct0
                    while jt < ct0 + nct:
                        w = int(tile_win[jt])
                        h = (jt - ct0) // (RCH // 128)
                        jend = min(jt + BLKT, wbound[w][1], ct0 + nct,
                                   ct0 + (h + 1) * (RCH // 128))
                        bw = (jend - jt) * 128
                        e0 = (jt - ct0) * 128             # chunk-local
                        eR = (jt - ct0) * 128 - h * RCH   # rch-local
                        lr_c, ea_c = rtiles[h]

                        prep = plp.tile([128, 512], FP, tag="plrep")
                        nc.tensor.matmul(prep[:, :bw], ones_s[:],
                                         lr_c[:, eR:eR + bw],
                                         start=True, stop=True)
                        oht = wk.tile([128, 512], BF, tag="oht")
                        nc.vector.tensor_tensor(oht[:, :bw], iop_s[:, :bw],
                                                prep[:, :bw], EQ)
                        ph_t = php.tile([128, 512], FP, tag="phid")
                        nc.tensor.matmul(ph_t[:, :bw],
                                         aloc_s[:, w * 128:(w + 1) * 128],
                                         oht[:, :bw], start=True, stop=False)
                        mi = nc.tensor.matmul(ph_t[:, :bw], wc_s[:],
                                              xt_c[:, 0, e0:e0 + bw],
                                              start=False, stop=False)
                        if xt_gate is not None:
                            mi.wait_op(xt_gate[0], xt_gate[1], "sem-ge")
                        nc.tensor.matmul(ph_t[:, :bw], we_s[:],
                                         ea_c[:, eR:eR + bw],
                                         start=False, stop=True)
                        silT = wk.tile([128, 512], BF, tag="silT")
                        nc.scalar.activation(silT[:, :bw], ph_t[:, :bw], SILU)
                        for t in range(jt, jend):
                            ctl = t - ct0
                            nc.tensor.matmul(
                                pg[:, ctl:ctl + 1],
                                silT[:, (t - jt) * 128:(t - jt + 1) * 128],
                                w2c_s[:], start=True, stop=True)
                        jt = jend

                    # -- batched sigmoid for the whole chunk
                    attn = wk.tile([128, CHUNK // 128, ], FP, tag="attn")
                    nc.scalar.activation(attn[:, :nct], pg[:, :nct], SIGM,
                                         bias=b2_s[:, 0:1])

                    # -- scatter pass
                    for t in range(ct0, ct0 + nct):
                        w = int(tile_win[t])
                        ws, wend = wbound[w]
                        ctl = t - ct0
                        gt = tbase + t
                        if t == ws:
                            pm_t[0] = pmp.tile([128, 128], FP, tag="pmsg",
                                               name="pmsg")
                        sel = wk.tile([128, 128], BF, tag="sel")
                        nc.vector.tensor_scalar(sel[:], iof_s[:],
                                                selc_s[:, gt:gt + 1],
                                                attn[:, ctl:ctl + 1], EQ, MUL)
                        mi = nc.tensor.matmul(pm_t[0][:], xm_c[:, ctl, :],
                                              sel[:], start=(t == ws),
                                              stop=(t == wend - 1))
                        if xm_gate is not None:
                            mi.wait_op(xm_gate[0], xm_gate[1], "sem-ge")
                        if t == wend - 1:
                            wblk = msg_acc[:, w * 128:(w + 1) * 128]
                            if tab_lo:
                                nc.vector.tensor_copy(wblk, pm_t[0][:])
                            else:
                                nc.vector.tensor_tensor(
                                    wblk, pm_t[0][:], wblk, ADD)

            for _p in (pmp, pgp, php, plp, chp):
                _p.release()

            # ---- out stage (two-pass, batched LN stats) ----------------
            ptp = tc.alloc_tile_pool(name="pt", bufs=2, space="PSUM")
            pop = tc.alloc_tile_pool(name="po", bufs=2, space="PSUM")
            s1buf = cp.tile([128, NLP], FP, tag="o_s1buf")
            musum = cp.tile([128, NW], FP, tag="o_musum")
            sqsum = cp.tile([128, NW], FP, tag="o_sqsum")
            mu2s = cp.tile([128, NW], FP, tag="o_mu2s")
            sq2s = cp.tile([128, NW], FP, tag="o_sq2s")

            for w in range(NW):
                wblk = slice(w * 128, (w + 1) * 128)
                mbf = wk.tile([128, 128], BF, tag="o_mbf")
                nc.vector.tensor_copy(mbf[:], msg_acc[:, wblk])
                po1 = pop.tile([128, 128], FP, tag="o_po1")
                nc.tensor.matmul(po1[:], mbf[:], w1o_s[:], start=True,
                                 stop=False)
                nc.tensor.matmul(po1[:], ones_s[:], b1o_s[:], start=False,
                                 stop=True)
                nc.scalar.activation(s1buf[:, wblk], po1[:], SILU,
                                     accum_out=musum[:, w:w + 1])
                jk = wk.tile([128, 128], BF, tag="o_jk")
                nc.vector.scalar_tensor_tensor(
                    jk[:], s1buf[:, wblk], 0.0, s1buf[:, wblk], BYP, MUL,
                    accum_out=sqsum[:, w:w + 1])

            def batch_stats(msum, sqs, tagp):
                mu = cp.tile([128, NW], FP, tag=f"{tagp}_mu")
                nc.vector.tensor_scalar(mu[:], msum[:], 1.0 / 128.0, None, MUL)
                musq = wk.tile([128, NW], FP, tag=f"{tagp}_musq")
                nc.vector.tensor_tensor(musq[:], mu[:], mu[:], MUL)
                var = wk.tile([128, NW], FP, tag=f"{tagp}_var")
                nc.vector.tensor_scalar(var[:], sqs[:], 1.0 / 128.0, None, MUL)
                var2 = wk.tile([128, NW], FP, tag=f"{tagp}_var2")
                nc.vector.tensor_tensor(var2[:], var[:], musq[:], SUB)
                std = wk.tile([128, NW], FP, tag=f"{tagp}_std")
                nc.scalar.activation(std[:], var2[:], SQRT, bias=eps_s[:, 0:1])
                rstd = cp.tile([128, NW], FP, tag=f"{tagp}_rstd")
                nc.vector.reciprocal(rstd[:], std[:])
                return mu, rstd

            mu1, rstd1 = batch_stats(musum, sqsum, "bs1")

            for w in range(NW):
                wblk = slice(w * 128, (w + 1) * 128)
                z = wk.tile([128, 128], BF, tag="o_z")
                nc.vector.tensor_scalar(z[:], s1buf[:, wblk], mu1[:, w:w + 1],
                                        rstd1[:, w:w + 1], SUB, MUL)
                pzt = ptp.tile([128, 128], BF, tag="o_pzt")
                nc.tensor.transpose(pzt[:], z[:], id_s[:])
                zt = wk.tile([128, 128], BF, tag="o_zt")
                nc.vector.tensor_copy(zt[:], pzt[:])
                po2 = pop.tile([128, 128], FP, tag="o_po2")
                nc.tensor.matmul(po2[:], zt[:], w2o_s[:], start=True,
                                 stop=False)
                nc.tensor.matmul(po2[:], ones_s[:], b2o_s[:], start=False,
                                 stop=True)
                # r = x + out_mlp(msg); overwrite s1buf window
                nc.vector.tensor_tensor(s1buf[:, wblk], po2[:],
                                        xres_s[:, wblk], ADD)
                nc.vector.reduce_sum(mu2s[:, w:w + 1], s1buf[:, wblk],
                                     axis=AXX)
                jk2 = wk.tile([128, 128], BF, tag="o_jk2")
                nc.vector.scalar_tensor_tensor(
                    jk2[:], s1buf[:, wblk], 0.0, s1buf[:, wblk], BYP, MUL,
                    accum_out=sq2s[:, w:w + 1])

            mu2, rstd2 = batch_stats(mu2s, sq2s, "bs2")

            for w in range(NW):
                wblk = slice(w * 128, (w + 1) * 128)
                zf = wk.tile([128, 128], FP, tag="o_zf")
                nc.vector.tensor_scalar(zf[:], s1buf[:, wblk],
                                        mu2[:, w:w + 1], rstd2[:, w:w + 1],
                                        SUB, MUL)
                zg = wk.tile([128, 128], FP, tag="o_zg")
                nc.vector.tensor_tensor(zg[:], zf[:], lngr_s[:], MUL)
                ot = wk.tile([128, 128], FP, tag="o_ot")
                nc.vector.tensor_tensor(ot[:], zg[:], lnbr_s[:], ADD)
                nc.sync.dma_start(out[:, wblk], ot[:])
            pop.release()
            ptp.release()

    nc.compile()
    return nc


def _phase2_inputs(plan, streams, p1_results, inp, inp2_b2):
    """Assemble per-core phase-2 in_maps from phase-1 outputs."""
    att_w1 = inp["att_w1"]
    att_w2 = inp["att_w2"]

    def unwrap(a, dt=F32):
        a = np.asarray(a, dtype=dt) if dt is not None else np.asarray(a)
        return a.reshape(128, NW, 128).transpose(1, 0, 2).reshape(NLP, 128)

    # global tables (bf16), padded to N rows only
    xtab = np.concatenate(
        [unwrap(p1_results[c]["x_out"])[:NL] for c in range(NCORES)])
    xmtab = np.concatenate(
        [unwrap(np.asarray(p1_results[c]["xm_out"], F32))[:NL]
         for c in range(NCORES)])
    xtab = _bf(xtab)
    xmtab = _bf(xmtab)

    w2op = inp["out_ln_g"][:, None] * inp["out_w2"]
    b2op = inp["out_ln_b"] @ inp["out_w2"] + inp["out_b2"]
    consts = {
        "xtab": xtab, "xmtab": xmtab,
        "iota_part": _f32(np.broadcast_to(np.arange(128)[:, None], (128, 512))),
        "iota_free": _f32(np.broadcast_to(np.arange(128)[None, :], (128, 128))),
        "wc": _bf(att_w1[D:2 * D]), "we_row": _bf(att_w1[2 * D:2 * D + 1]),
        "w2col": _bf(att_w2[:, 0:1]),
        "ident": _bf(np.eye(128)), "ones1": _bf(np.ones((1, 128))),
        "w1o": _bf(inp["out_w1"]), "b1o_row": _bf(inp["out_b1"][None, :]),
        "w2op": _bf(w2op), "b2op_row": _bf(b2op[None, :]),
        "lng_rep": _f32(np.broadcast_to(inp["ln_g"][None, :], (128, 128))),
        "lnb_rep": _f32(np.broadcast_to(inp["ln_b"][None, :], (128, 128))),
        "eps_col": _f32(np.full((128, 1), LN_EPS)),
        "b2_col": _f32(np.full((128, 1), inp2_b2)),
    }
    in_maps = []
    for c in range(NCORES):
        in_maps.append({
            "aloc": np.ascontiguousarray(np.asarray(p1_results[c]["a_out"])),
            "xres": np.ascontiguousarray(np.asarray(p1_results[c]["x_out"])),
            "sel_col": streams["sel_col"][c],
            "idx": streams["idx"][c],
            "lrow_row": streams["lrow_row"][c],
            "ea_row": streams["ea_row"][c],
            **consts,
        })
    return in_maps


def kernel(**inputs):
    inp = {k: np.asarray(v) for k, v in inputs.items()}
    h = _f32(inp["h"])
    weights = (inp["lin_w"], inp["lin_b"], inp["msg_w1"], inp["msg_b1"],
               inp["msg_ln_g"], inp["msg_ln_b"], inp["msg_w2"], inp["msg_b2"],
               inp["att_w1"], inp["att_b1"])
    b2 = float(np.asarray(inp["att_b2"]).ravel()[0])
    plan, streams = _prep_edges(inp["row"], inp["col"], inp["edge_attr"],
                                inp["edge_mask"])

    p1 = _run_phase1(h, weights)

    key = (tuple(plan["WL"]), tuple(plan["WH"]))
    if _cache.get("p2_key") != key:
        _cache["p2"] = _build_phase2(plan)
        _cache["p2_key"] = key
    nc2 = _cache["p2"]
    in_maps = _phase2_inputs(plan, streams, p1, inp, b2)
    res = run_bass_kernel_spmd(nc2, in_maps, core_ids=list(range(NCORES)),
                               trace=TRACE)
    if TRACE:
        LAST_RESULTS["phase2_ns"] = res.exec_time_ns
    out = np.concatenate([
        np.asarray(res.results[c]["out"], F32)
        .reshape(128, NW, 128).transpose(1, 0, 2).reshape(NLP, 128)[:NL]
        for c in range(NCORES)])
    return out.astype(F32)


# revision 26
# speedup vs baseline: 1.0670x; 1.0664x over previous
"""GCLayer (GNN message passing) on 8 Trainium2 NeuronCores.

Strategy
--------
Edges are partitioned by destination row (node-range sharding): core c owns
nodes [c*6250, (c+1)*6250) and every edge whose `row` lands there, so the
segment-sum needs no cross-core reduction.

Phase 1 (node stage, sharded): x = h@lin_w+b, x_ = msg_mlp(x),
A = x@att_w1[:D]+att_b1 per local node (row-major).  The host assembles the
global x / x_ tables (bf16) and feeds them to every core.

Phase 2 (edge + out stage), feature-major ("flipped") attention:
  hid^T[j, e] = A^T-gather (aloc lhsT @ one-hot rhs) + wc lhsT @ x^T[col]
                + we outer ea           (PSUM, blocks up to 512 edges wide)
  sil^T = SILU(hid^T)                   (ACT, one op per block)
  logit[e] = sil^T slice (lhsT) @ w2col (PE does the w2 dot, one col/tile)
  att = sigmoid(logits + b2)            (ACT, ONE op per 64-tile chunk ->
                                         no per-tile SILU/SIGMOID table churn)
  sel[e, s] = att_e * (iota_f == lrow_e) (vector tensor_scalar, fused)
  msg^T[d, s] += xm[col] (lhsT) @ sel   (PE scatter, transposed msg)
Masking/padding is exact via a sentinel lrow (=200) so sel columns of pad or
masked edges are all-zero; no -30000 bias hack needed.

x^T[col] / xm[col] are fetched with SWDGE dma_gather in prepare_only mode +
trigger_dma on 4 SWDGE queues, so the gpsimd engine only pays descriptor
generation and the random-access HBM transfers overlap compute.

LayerNorms (phase 1, out stage) are two-pass with batched stats: SILU keeps
its ACT table resident; Var = E[x^2]-mu^2 via scalar_tensor_tensor accum;
sqrt over all 49 windows in one ACT op.
"""

import sys

sys.path.insert(0, "/opt/trn_rl_repo")

import numpy as np
import ml_dtypes

from concourse import bacc, mybir, tile
from concourse.bass_utils import run_bass_kernel_spmd

BF16 = ml_dtypes.bfloat16
F32 = np.float32

NCORES = 8
N = 50000
E = 800000
D = 128
NL = N // NCORES          # 6250 real nodes per core
NW = 49                   # node blocks of 128 per core (49*128 = 6272)
NLP = NW * 128            # padded nodes per core
SPLIT = 32768             # int16 gather index limit
CHUNK = 8192              # edges per dma_gather call (64 tiles)
RCH = 2048                # row-stream load granularity (16 tiles)
BLKT = 4                  # tiles per hidden block (<=512 edges)
PREP_GATHER = True        # prepare_only + trigger_dma (overlapped transfers)
LN_EPS = 1e-5
SENT = 200.0              # sentinel lrow for pad/masked edges (> 127)

FP = mybir.dt.float32
BF = mybir.dt.bfloat16
I16 = mybir.dt.int16

TRACE = False             # test.py sets kernel.TRACE = True for profiling
LAST_RESULTS = {}         # exec_time_ns per phase when TRACE

_cache = {}


def _bf(a):
    return np.ascontiguousarray(np.asarray(a, dtype=F32).astype(BF16))


def _f32(a):
    return np.ascontiguousarray(np.asarray(a, dtype=F32))


def _ceil(a, m):
    return -(-int(a) // m) * m


# ---------------------------------------------------------------------------
# Host-side edge preprocessing
# ---------------------------------------------------------------------------

def _prep_edges(row, col, ea, em):
    """Sort/partition/pad edges. Returns per-core streams + the static plan."""
    row = np.asarray(row).astype(np.int64).ravel()
    col = np.asarray(col).astype(np.int64).ravel()
    ea = np.asarray(ea, dtype=F32).ravel()
    em = np.asarray(em, dtype=F32).ravel()

    c_of = row // NL
    r_loc = row - c_of * NL
    w_of = r_loc // 128
    s_in_w = r_loc % 128
    hi = (col >= SPLIT).astype(np.int64)

    # bucket = (core, pass(lo/hi), window); argsort gives the stream order
    key = (c_of * 2 + hi) * NW + w_of
    order = np.argsort(key, kind="stable")
    skey = key[order]
    cnt = np.bincount(key, minlength=NCORES * 2 * NW).reshape(NCORES, 2, NW)

    WL = np.array([_ceil(cnt[:, 0, w].max(), 128) for w in range(NW)])
    WH = np.array([_ceil(cnt[:, 1, w].max(), 128) for w in range(NW)])
    lo_total = int(WL.sum())
    hi_total = int(WH.sum())
    EP = lo_total + hi_total

    # padded base offset of each (pass, window) block within the stream
    sizes = np.concatenate([WL, WH])                      # (2*NW,)
    base = np.concatenate([[0], np.cumsum(sizes)[:-1]])   # (2*NW,)

    # rank of each edge within its bucket
    bstart = np.concatenate([[0], np.cumsum(cnt.ravel())[:-1]])
    rank = np.arange(row.size) - bstart[skey]
    pw = skey % (2 * NW)                                   # (pass, window) id
    dest = base[pw] + rank                                 # position in stream
    cc = skey // (2 * NW)

    g_lrow = np.zeros((NCORES, EP), F32)
    g_sel = np.full((NCORES, EP), SENT, F32)   # sentinel: pad edges -> sel 0
    g_idx = np.zeros((NCORES, EP), np.int16)
    g_ea = np.zeros((NCORES, EP), F32)

    e_ids = order
    g_lrow[cc, dest] = s_in_w[e_ids].astype(F32)
    # masked real edges keep the sentinel too (exact edge_mask handling)
    lsel = np.where(em[e_ids] > 0.5, s_in_w[e_ids].astype(F32), SENT)
    g_sel[cc, dest] = lsel
    g_idx[cc, dest] = (col[e_ids] - hi[e_ids] * SPLIT).astype(np.int16)
    g_ea[cc, dest] = ea[e_ids]

    # wrapped layouts
    sel_col = np.ascontiguousarray(
        g_sel.reshape(NCORES, EP // 128, 128).transpose(0, 2, 1))
    idx16 = g_idx.reshape(NCORES, EP // 16, 16).transpose(0, 2, 1)  # (8,16,EP/16)
    idx_w = np.ascontiguousarray(np.tile(idx16, (1, 8, 1)))         # (8,128,EP/16)
    lrow_row = _bf(g_lrow.reshape(NCORES, 1, EP))
    ea_row = _bf(g_ea.reshape(NCORES, 1, EP))

    def chunks(total, start):
        out = []
        off = 0
        while off < total:
            g = min(CHUNK, total - off)
            out.append((start + off, g))
            off += g
        return out

    plan = dict(
        WL=[int(x) for x in WL], WH=[int(x) for x in WH], EP=EP,
        lo_total=lo_total, hi_total=hi_total,
        chunks_lo=chunks(lo_total, 0), chunks_hi=chunks(hi_total, lo_total),
    )
    streams = dict(sel_col=sel_col, idx=idx_w,
                   lrow_row=lrow_row, ea_row=ea_row)
    return plan, streams


# ---------------------------------------------------------------------------
# Phase 1: node stage (sharded over nodes)
# ---------------------------------------------------------------------------

def _build_phase1():
    nc = bacc.Bacc("TRN2", target_bir_lowering=False, debug=False,
                   num_devices=NCORES)
    g = lambda n, s, d, k: nc.dram_tensor(n, s, d, kind=k).ap()

    ht = g("ht", [128, NLP], BF, "ExternalInput")          # h^T, node-wrapped
    linw = g("linw", [128, 128], BF, "ExternalInput")
    linb = g("linb_row", [1, 128], BF, "ExternalInput")
    w1m = g("w1m", [128, 128], BF, "ExternalInput")
    b1m = g("b1m_row", [1, 128], BF, "ExternalInput")
    w2mp = g("w2mp", [128, 128], BF, "ExternalInput")
    b2mp = g("b2mp_row", [1, 128], BF, "ExternalInput")
    wr = g("wr", [128, 128], BF, "ExternalInput")
    b1a = g("b1a_row", [1, 128], BF, "ExternalInput")
    ident = g("ident", [128, 128], BF, "ExternalInput")
    ones1 = g("ones1", [1, 128], BF, "ExternalInput")

    eps_c = g("eps_col", [128, 1], FP, "ExternalInput")

    x_out = g("x_out", [128, NLP], FP, "ExternalOutput")
    xm_out = g("xm_out", [128, NLP], BF, "ExternalOutput")
    a_out = g("a_out", [128, NLP], BF, "ExternalOutput")

    SILU = mybir.ActivationFunctionType.Silu
    SQRT = mybir.ActivationFunctionType.Sqrt
    MUL = mybir.AluOpType.mult
    SUB = mybir.AluOpType.subtract
    BYP = mybir.AluOpType.bypass

    with tile.TileContext(nc) as tc:
        with tc.tile_pool(name="const", bufs=1) as cp, \
             tc.tile_pool(name="work", bufs=3) as wp, \
             tc.tile_pool(name="psum", bufs=1, space="PSUM") as pp, \
             tc.tile_pool(name="psum2", bufs=2, space="PSUM") as pp2:

            def cload(ap, shape, dt, tag):
                t = cp.tile(shape, dt, tag=tag)
                nc.sync.dma_start(t[:], ap)
                return t

            ht_s = cload(ht, [128, NLP], BF, "c_ht")
            linw_s = cload(linw, [128, 128], BF, "c_linw")
            linb_s = cload(linb, [1, 128], BF, "c_linb")
            w1m_s = cload(w1m, [128, 128], BF, "c_w1m")
            b1m_s = cload(b1m, [1, 128], BF, "c_b1m")
            w2mp_s = cload(w2mp, [128, 128], BF, "c_w2mp")
            b2mp_s = cload(b2mp, [1, 128], BF, "c_b2mp")
            wr_s = cload(wr, [128, 128], BF, "c_wr")
            b1a_s = cload(b1a, [1, 128], BF, "c_b1a")
            id_s = cload(ident, [128, 128], BF, "c_id")
            ones_s = cload(ones1, [1, 128], BF, "c_ones")
            eps_s = cload(eps_c, [128, 1], FP, "c_eps")

            s1buf = cp.tile([128, NLP], FP, tag="s1buf")
            musum = cp.tile([128, NW], FP, tag="musum")
            sqsum = cp.tile([128, NW], FP, tag="sqsum")
            xtbuf = cp.tile([128, NLP], BF, tag="xtbuf")

            # ---- pass A: x, x^T, A, s1 = silu(mlp1(x)), LN stats ---------
            for w in range(NW):
                blk = slice(w * 128, (w + 1) * 128)
                htb = ht_s[:, blk]

                # x = h @ lin_w + lin_b      [n, d]  (row-major)
                px = pp.tile([128, 128], FP, tag="px")
                nc.tensor.matmul(px[:], htb, linw_s[:], start=True, stop=False)
                nc.tensor.matmul(px[:], ones_s[:], linb_s[:], start=False,
                                 stop=True)
                xs = wp.tile([128, 128], FP, tag="xs")
                nc.scalar.copy(xs[:], px[:])
                nc.sync.dma_start(x_out[:, blk], xs[:])

                # x^T  (for downstream lhsT use)
                pxt = pp.tile([128, 128], FP, tag="pxt")
                nc.tensor.matmul(pxt[:], linw_s[:], htb, start=True, stop=False)
                nc.tensor.matmul(pxt[:], linb_s[:], ones_s[:], start=False,
                                 stop=True)
                nc.vector.tensor_copy(xtbuf[:, blk], pxt[:])

                # A = x @ wr + b1a           [n, h]  (row-major)
                pa = pp.tile([128, 128], FP, tag="pa")
                nc.tensor.matmul(pa[:], xtbuf[:, blk], wr_s[:], start=True,
                                 stop=False)
                nc.tensor.matmul(pa[:], ones_s[:], b1a_s[:], start=False,
                                 stop=True)
                asb = wp.tile([128, 128], BF, tag="asb")
                nc.scalar.copy(asb[:], pa[:])
                nc.sync.dma_start(a_out[:, blk], asb[:])

                # s1 = silu(x @ w1m + b1m), musum = sum(s1)
                ps = pp.tile([128, 128], FP, tag="ps")
                nc.tensor.matmul(ps[:], xtbuf[:, blk], w1m_s[:], start=True,
                                 stop=False)
                nc.tensor.matmul(ps[:], ones_s[:], b1m_s[:], start=False,
                                 stop=True)
                nc.scalar.activation(s1buf[:, blk], ps[:], SILU,
                                     accum_out=musum[:, w:w + 1])
                jk = wp.tile([128, 128], BF, tag="jk")
                nc.vector.scalar_tensor_tensor(
                    jk[:], s1buf[:, blk], 0.0, s1buf[:, blk], BYP, MUL,
                    accum_out=sqsum[:, w:w + 1])

            # ---- batched LN stats ---------------------------------------
            mu = cp.tile([128, NW], FP, tag="mu")
            nc.vector.tensor_scalar(mu[:], musum[:], 1.0 / 128.0, None, MUL)
            musq = wp.tile([128, NW], FP, tag="musq")
            nc.vector.tensor_tensor(musq[:], mu[:], mu[:], MUL)
            var = wp.tile([128, NW], FP, tag="var")
            nc.vector.tensor_scalar(var[:], sqsum[:], 1.0 / 128.0, None, MUL)
            var2 = wp.tile([128, NW], FP, tag="var2")
            nc.vector.tensor_tensor(var2[:], var[:], musq[:], SUB)
            std = wp.tile([128, NW], FP, tag="std")
            nc.scalar.activation(std[:], var2[:], SQRT, bias=eps_s[:, 0:1])
            rstd = cp.tile([128, NW], FP, tag="rstd")
            nc.vector.reciprocal(rstd[:], std[:])

            # ---- pass B: z -> x_ = z @ w2mp + b2mp ----------------------
            for w in range(NW):
                blk = slice(w * 128, (w + 1) * 128)
                z = wp.tile([128, 128], BF, tag="z")
                nc.vector.tensor_scalar(z[:], s1buf[:, blk], mu[:, w:w + 1],
                                        rstd[:, w:w + 1], SUB, MUL)
                pzt = pp2.tile([128, 128], BF, tag="pzt")
                nc.tensor.transpose(pzt[:], z[:], id_s[:])
                zt = wp.tile([128, 128], BF, tag="zt")
                nc.vector.tensor_copy(zt[:], pzt[:])
                pxm = pp.tile([128, 128], FP, tag="pxm")
                nc.tensor.matmul(pxm[:], zt[:], w2mp_s[:], start=True,
                                 stop=False)
                nc.tensor.matmul(pxm[:], ones_s[:], b2mp_s[:], start=False,
                                 stop=True)
                xm = wp.tile([128, 128], BF, tag="xm")
                nc.scalar.copy(xm[:], pxm[:])
                nc.sync.dma_start(xm_out[:, blk], xm[:])

    nc.compile()
    return nc


def _phase1_inputs(h, weights):
    """Per-core in_maps for phase 1."""
    (lin_w, lin_b, msg_w1, msg_b1, msg_ln_g, msg_ln_b, msg_w2, msg_b2,
     att_w1, att_b1) = weights
    w2mp = msg_ln_g[:, None] * msg_w2
    b2mp = msg_ln_b @ msg_w2 + msg_b2
    consts = {
        "linw": _bf(lin_w), "linb_row": _bf(lin_b[None, :]),
        "w1m": _bf(msg_w1), "b1m_row": _bf(msg_b1[None, :]),
        "w2mp": _bf(w2mp), "b2mp_row": _bf(b2mp[None, :]),
        "wr": _bf(att_w1[0:D]), "b1a_row": _bf(att_b1[None, :]),
        "ident": _bf(np.eye(128)), "ones1": _bf(np.ones((1, 128))),
        "eps_col": _f32(np.full((128, 1), LN_EPS)),
    }
    in_maps = []
    for c in range(NCORES):
        hc = np.zeros((NLP, D), F32)
        hc[:NL] = h[c * NL:(c + 1) * NL]
        in_maps.append({"ht": _bf(hc.T), **consts})
    return in_maps


def _run_phase1(h, weights):
    if "p1" not in _cache:
        _cache["p1"] = _build_phase1()
    nc = _cache["p1"]
    in_maps = _phase1_inputs(h, weights)
    res = run_bass_kernel_spmd(nc, in_maps, core_ids=list(range(NCORES)),
                               trace=TRACE)
    if TRACE:
        LAST_RESULTS["phase1_ns"] = res.exec_time_ns
    return res.results


# ---------------------------------------------------------------------------
# Phase 2: edge stage (gather/attention/scatter) + out stage
# ---------------------------------------------------------------------------

def _build_phase2(plan):
    WL, WH, EP = plan["WL"], plan["WH"], plan["EP"]
    chunks_lo, chunks_hi = plan["chunks_lo"], plan["chunks_hi"]
    lo_tiles = plan["lo_total"] // 128

    nc = bacc.Bacc("TRN2", target_bir_lowering=False, debug=False,
                   num_devices=NCORES, num_swdge_queues=4)
    g = lambda n, s, d, k: nc.dram_tensor(n, s, d, kind=k).ap()

    xtab = g("xtab", [N, 128], BF, "ExternalInput")
    xmtab = g("xmtab", [N, 128], BF, "ExternalInput")
    aloc = g("aloc", [128, NLP], BF, "ExternalInput")      # A rows per window
    xres = g("xres", [128, NLP], FP, "ExternalInput")      # x rows per window
    selc = g("sel_col", [128, EP // 128], FP, "ExternalInput")
    idxt = g("idx", [128, EP // 16], I16, "ExternalInput")
    lrowr = g("lrow_row", [1, EP], BF, "ExternalInput")
    ear = g("ea_row", [1, EP], BF, "ExternalInput")

    iota_p = g("iota_part", [128, 512], FP, "ExternalInput")
    iota_f = g("iota_free", [128, 128], FP, "ExternalInput")
    wc = g("wc", [128, 128], BF, "ExternalInput")
    we = g("we_row", [1, 128], BF, "ExternalInput")
    w2c = g("w2col", [128, 1], BF, "ExternalInput")
    ident = g("ident", [128, 128], BF, "ExternalInput")
    ones1 = g("ones1", [1, 128], BF, "ExternalInput")
    w1o = g("w1o", [128, 128], BF, "ExternalInput")
    b1o = g("b1o_row", [1, 128], BF, "ExternalInput")
    w2o = g("w2op", [128, 128], BF, "ExternalInput")
    b2o = g("b2op_row", [1, 128], BF, "ExternalInput")
    lngr = g("lng_rep", [128, 128], FP, "ExternalInput")
    lnbr = g("lnb_rep", [128, 128], FP, "ExternalInput")
    eps_c = g("eps_col", [128, 1], FP, "ExternalInput")
    b2c = g("b2_col", [128, 1], FP, "ExternalInput")

    out = g("out", [128, NLP], FP, "ExternalOutput")

    SILU = mybir.ActivationFunctionType.Silu
    SIGM = mybir.ActivationFunctionType.Sigmoid
    SQRT = mybir.ActivationFunctionType.Sqrt
    EQ = mybir.AluOpType.is_equal
    MUL = mybir.AluOpType.mult
    ADD = mybir.AluOpType.add
    SUB = mybir.AluOpType.subtract
    BYP = mybir.AluOpType.bypass
    AXX = mybir.AxisListType.X

    with tile.TileContext(nc) as tc:
        with tc.tile_pool(name="const", bufs=1) as cp, \
             tc.tile_pool(name="stream", bufs=1) as sp, \
             tc.tile_pool(name="work", bufs=3) as wk:

            def cload(ap, shape, dt, tag, pool=None):
                t = (pool or cp).tile(shape, dt, tag=tag)
                nc.sync.dma_start(t[:], ap)
                return t

            aloc_s = cload(aloc, [128, NLP], BF, "c_aloc")
            selc_s = cload(selc, [128, EP // 128], FP, "c_selc", sp)
            iop_s = cload(iota_p, [128, 512], FP, "c_iop")
            iof_s = cload(iota_f, [128, 128], FP, "c_iof")
            wc_s = cload(wc, [128, 128], BF, "c_wc")
            we_s = cload(we, [1, 128], BF, "c_we")
            w2c_s = cload(w2c, [128, 1], BF, "c_w2c")
            id_s = cload(ident, [128, 128], BF, "c_id")
            ones_s = cload(ones1, [1, 128], BF, "c_ones")
            w1o_s = cload(w1o, [128, 128], BF, "c_w1o")
            b1o_s = cload(b1o, [1, 128], BF, "c_b1o")
            w2o_s = cload(w2o, [128, 128], BF, "c_w2o")
            b2o_s = cload(b2o, [1, 128], BF, "c_b2o")
            lngr_s = cload(lngr, [128, 128], FP, "c_lngr")
            lnbr_s = cload(lnbr, [128, 128], FP, "c_lnbr")
            eps_s = cload(eps_c, [128, 1], FP, "c_eps")
            b2_s = cload(b2c, [128, 1], FP, "c_b2")

            msg_acc = cp.tile([128, NLP], FP, tag="msg_acc")   # msg^T [d, s]

            dma_sems = [nc.alloc_semaphore(f"swdge_dma{q}") for q in range(4)]

            # ---- edge passes -------------------------------------------
            chp = tc.alloc_tile_pool(name="chunk", bufs=2)
            xmp = tc.alloc_tile_pool(name="xmchunk", bufs=3)
            plp = tc.alloc_tile_pool(name="pl", bufs=2, space="PSUM")
            php = tc.alloc_tile_pool(name="ph", bufs=2, space="PSUM")
            pgp = tc.alloc_tile_pool(name="pg", bufs=2, space="PSUM")
            pmp = tc.alloc_tile_pool(name="pm", bufs=2, space="PSUM")
            qn = [0]
            qcnt = [0, 0, 0, 0]

            for pi, (wsizes, chunks, tbase, tab_lo) in enumerate([
                    (WL, chunks_lo, 0, True), (WH, chunks_hi, lo_tiles, False)]):
                if tab_lo:
                    xt_src, xm_src = xtab[0:SPLIT], xmtab[0:SPLIT]
                else:
                    xt_src, xm_src = xtab[SPLIT:N], xmtab[SPLIT:N]
                pass_start = chunks[0][0]

                # window boundaries in pass-tile coordinates
                wbound = []
                j = 0
                for w in range(NW):
                    nt = wsizes[w] // 128
                    wbound.append((j, j + nt))
                    j += nt
                    if nt == 0 and tab_lo:
                        nc.vector.memset(
                            msg_acc[:, w * 128:(w + 1) * 128], 0.0)
                tile_win = np.zeros(j, np.int64)
                for w, (a, b) in enumerate(wbound):
                    tile_win[a:b] = w

                pm_t = [None]

                for ci, (off, gsz) in enumerate(chunks):
                    nct = gsz // 128
                    ct0 = (off - pass_start) // 128    # first pass-tile

                    # -- gathers: prepare_only + trigger on rotating queues
                    idx_c = chp.tile([128, CHUNK // 16], I16, tag="cidx")
                    nc.sync.dma_start(idx_c[:, :gsz // 16],
                                      idxt[:, off // 16:(off + gsz) // 16])
                    xt_c = chp.tile([128, 1, CHUNK], BF, tag="cxT")
                    xm_c = xmp.tile([128, CHUNK // 128, 128], BF, tag="cxm")
                    xt_gate = xm_gate = None
                    if PREP_GATHER:
                        q = qn[0] % 4
                        nc.gpsimd.dma_gather(
                            xt_c[:, :, :gsz], xt_src,
                            idx_c[:, :gsz // 16],
                            gsz, gsz, 128, transpose=True,
                            single_packet=False, prepare_only=True,
                            sem=dma_sems[q], queue_num=q)
                        nc.gpsimd.trigger_dma(count=None, queue_num=q)
                        qcnt[q] += 1
                        xt_gate = (dma_sems[q], 16 * qcnt[q])
                        q = (qn[0] + 1) % 4
                        nc.gpsimd.dma_gather(
                            xm_c[:, :gsz // 128, :], xm_src,
                            idx_c[:, :gsz // 16],
                            gsz, gsz, 128, single_packet=False,
                            prepare_only=True, sem=dma_sems[q], queue_num=q)
                        nc.gpsimd.trigger_dma(count=None, queue_num=q)
                        qcnt[q] += 1
                        xm_gate = (dma_sems[q], 16 * qcnt[q])
                        qn[0] += 2
                    else:
                        nc.gpsimd.dma_gather(
                            xt_c[:, :, :gsz], xt_src,
                            idx_c[:, :gsz // 16],
                            gsz, gsz, 128, transpose=True,
                            single_packet=False)
                        nc.gpsimd.dma_gather(
                            xm_c[:, :gsz // 128, :], xm_src,
                            idx_c[:, :gsz // 16],
                            gsz, gsz, 128, single_packet=False)

                    # -- row streams (lrow/ea) in RCH slices
                    rtiles = {}
                    for h in range((gsz + RCH - 1) // RCH):
                        roff = off + h * RCH
                        rsz = min(RCH, off + gsz - roff)
                        lr_c = chp.tile([1, RCH], BF, tag="crow")
                        nc.sync.dma_start(lr_c[:, :rsz],
                                          lrowr[0:1, roff:roff + rsz])
                        ea_c = chp.tile([1, RCH], BF, tag="cea")
                        nc.sync.dma_start(ea_c[:, :rsz],
                                          ear[0:1, roff:roff + rsz])
                        rtiles[h] = (lr_c, ea_c)

                    # -- compute pass: blocks of <= BLKT tiles
                    pg = pgp.tile([128, CHUNK // 128], FP, tag="pgl")
                    jt = ct0
                    while jt < ct0 + nct:
                        w = int(tile_win[jt])
                        h = (jt - ct0) // (RCH // 128)
                        jend = min(jt + BLKT, wbound[w][1], ct0 + nct,
                                   ct0 + (h + 1) * (RCH // 128))
                        bw = (jend - jt) * 128
                        e0 = (jt - ct0) * 128             # chunk-local
                        eR = (jt - ct0) * 128 - h * RCH   # rch-local
                        lr_c, ea_c = rtiles[h]

                        prep = plp.tile([128, 512], FP, tag="plrep")
                        nc.tensor.matmul(prep[:, :bw], ones_s[:],
                                         lr_c[:, eR:eR + bw],
                                         start=True, stop=True)
                        oht = wk.tile([128, 512], BF, tag="oht")
                        nc.vector.tensor_tensor(oht[:, :bw], iop_s[:, :bw],
                                                prep[:, :bw], EQ)
                        ph_t = php.tile([128, 512], FP, tag="phid")
                        nc.tensor.matmul(ph_t[:, :bw],
                                         aloc_s[:, w * 128:(w + 1) * 128],
                                         oht[:, :bw], start=True, stop=False)
                        mi = nc.tensor.matmul(ph_t[:, :bw], wc_s[:],
                                              xt_c[:, 0, e0:e0 + bw],
                                              start=False, stop=False)
                        if xt_gate is not None:
                            mi.wait_op(xt_gate[0], xt_gate[1], "sem-ge")
                        nc.tensor.matmul(ph_t[:, :bw], we_s[:],
                                         ea_c[:, eR:eR + bw],
                                         start=False, stop=True)
                        silT = wk.tile([128, 512], BF, tag="silT")
                        nc.scalar.activation(silT[:, :bw], ph_t[:, :bw], SILU)
                        for t in range(jt, jend):
                            ctl = t - ct0
                            nc.tensor.matmul(
                                pg[:, ctl:ctl + 1],
                                silT[:, (t - jt) * 128:(t - jt + 1) * 128],
                                w2c_s[:], start=True, stop=True)
                        jt = jend

                    # -- batched sigmoid for the whole chunk
                    attn = wk.tile([128, CHUNK // 128, ], FP, tag="attn")
                    nc.scalar.activation(attn[:, :nct], pg[:, :nct], SIGM,
                                         bias=b2_s[:, 0:1])

                    # -- scatter pass
                    for t in range(ct0, ct0 + nct):
                        w = int(tile_win[t])
                        ws, wend = wbound[w]
                        ctl = t - ct0
                        gt = tbase + t
                        if t == ws:
                            pm_t[0] = pmp.tile([128, 128], FP, tag="pmsg",
                                               name="pmsg")
                        sel = wk.tile([128, 128], BF, tag="sel")
                        nc.vector.tensor_scalar(sel[:], iof_s[:],
                                                selc_s[:, gt:gt + 1],
                                                attn[:, ctl:ctl + 1], EQ, MUL)
                        mi = nc.tensor.matmul(pm_t[0][:], xm_c[:, ctl, :],
                                              sel[:], start=(t == ws),
                                              stop=(t == wend - 1))
                        if xm_gate is not None:
                            mi.wait_op(xm_gate[0], xm_gate[1], "sem-ge")
                        if t == wend - 1:
                            wblk = msg_acc[:, w * 128:(w + 1) * 128]
                            if tab_lo:
                                nc.vector.tensor_copy(wblk, pm_t[0][:])
                            else:
                                nc.vector.tensor_tensor(
                                    wblk, pm_t[0][:], wblk, ADD)

            for _p in (pmp, pgp, php, plp, xmp, chp):
                _p.release()

            # ---- out stage (two-pass, batched LN stats) ----------------
            ptp = tc.alloc_tile_pool(name="pt", bufs=2, space="PSUM")
            pop = tc.alloc_tile_pool(name="po", bufs=2, space="PSUM")
            xrp = tc.alloc_tile_pool(name="xresp", bufs=1)
            xres_s = xrp.tile([128, NLP], FP, tag="c_xres", name="c_xres")
            nc.sync.dma_start(xres_s[:], xres)
            s1buf = cp.tile([128, NLP], FP, tag="o_s1buf")
            musum = cp.tile([128, NW], FP, tag="o_musum")
            sqsum = cp.tile([128, NW], FP, tag="o_sqsum")
            mu2s = cp.tile([128, NW], FP, tag="o_mu2s")
            sq2s = cp.tile([128, NW], FP, tag="o_sq2s")

            for w in range(NW):
                wblk = slice(w * 128, (w + 1) * 128)
                mbf = wk.tile([128, 128], BF, tag="o_mbf")
                nc.vector.tensor_copy(mbf[:], msg_acc[:, wblk])
                po1 = pop.tile([128, 128], FP, tag="o_po1")
                nc.tensor.matmul(po1[:], mbf[:], w1o_s[:], start=True,
                                 stop=False)
                nc.tensor.matmul(po1[:], ones_s[:], b1o_s[:], start=False,
                                 stop=True)
                nc.scalar.activation(s1buf[:, wblk], po1[:], SILU,
                                     accum_out=musum[:, w:w + 1])
                jk = wk.tile([128, 128], BF, tag="o_jk")
                nc.vector.scalar_tensor_tensor(
                    jk[:], s1buf[:, wblk], 0.0, s1buf[:, wblk], BYP, MUL,
                    accum_out=sqsum[:, w:w + 1])

            def batch_stats(msum, sqs, tagp):
                mu = cp.tile([128, NW], FP, tag=f"{tagp}_mu")
                nc.vector.tensor_scalar(mu[:], msum[:], 1.0 / 128.0, None, MUL)
                musq = wk.tile([128, NW], FP, tag=f"{tagp}_musq")
                nc.vector.tensor_tensor(musq[:], mu[:], mu[:], MUL)
                var = wk.tile([128, NW], FP, tag=f"{tagp}_var")
                nc.vector.tensor_scalar(var[:], sqs[:], 1.0 / 128.0, None, MUL)
                var2 = wk.tile([128, NW], FP, tag=f"{tagp}_var2")
                nc.vector.tensor_tensor(var2[:], var[:], musq[:], SUB)
                std = wk.tile([128, NW], FP, tag=f"{tagp}_std")
                nc.scalar.activation(std[:], var2[:], SQRT, bias=eps_s[:, 0:1])
                rstd = cp.tile([128, NW], FP, tag=f"{tagp}_rstd")
                nc.vector.reciprocal(rstd[:], std[:])
                return mu, rstd

            mu1, rstd1 = batch_stats(musum, sqsum, "bs1")

            for w in range(NW):
                wblk = slice(w * 128, (w + 1) * 128)
                z = wk.tile([128, 128], BF, tag="o_z")
                nc.vector.tensor_scalar(z[:], s1buf[:, wblk], mu1[:, w:w + 1],
                                        rstd1[:, w:w + 1], SUB, MUL)
                pzt = ptp.tile([128, 128], BF, tag="o_pzt")
                nc.tensor.transpose(pzt[:], z[:], id_s[:])
                zt = wk.tile([128, 128], BF, tag="o_zt")
                nc.vector.tensor_copy(zt[:], pzt[:])
                po2 = pop.tile([128, 128], FP, tag="o_po2")
                nc.tensor.matmul(po2[:], zt[:], w2o_s[:], start=True,
                                 stop=False)
                nc.tensor.matmul(po2[:], ones_s[:], b2o_s[:], start=False,
                                 stop=True)
                # r = x + out_mlp(msg); overwrite s1buf window
                nc.vector.tensor_tensor(s1buf[:, wblk], po2[:],
                                        xres_s[:, wblk], ADD)
                nc.vector.reduce_sum(mu2s[:, w:w + 1], s1buf[:, wblk],
                                     axis=AXX)
                jk2 = wk.tile([128, 128], BF, tag="o_jk2")
                nc.vector.scalar_tensor_tensor(
                    jk2[:], s1buf[:, wblk], 0.0, s1buf[:, wblk], BYP, MUL,
                    accum_out=sq2s[:, w:w + 1])

            mu2, rstd2 = batch_stats(mu2s, sq2s, "bs2")

            for w in range(NW):
                wblk = slice(w * 128, (w + 1) * 128)
                zf = wk.tile([128, 128], FP, tag="o_zf")
                nc.vector.tensor_scalar(zf[:], s1buf[:, wblk],
                                        mu2[:, w:w + 1], rstd2[:, w:w + 1],
                                        SUB, MUL)
                zg = wk.tile([128, 128], FP, tag="o_zg")
                nc.vector.tensor_tensor(zg[:], zf[:], lngr_s[:], MUL)
                ot = wk.tile([128, 128], FP, tag="o_ot")
                nc.vector.tensor_tensor(ot[:], zg[:], lnbr_s[:], ADD)
                nc.sync.dma_start(out[:, wblk], ot[:])
            pop.release()
            ptp.release()
            xrp.release()

    nc.compile()
    return nc


def _phase2_inputs(plan, streams, p1_results, inp, inp2_b2):
    """Assemble per-core phase-2 in_maps from phase-1 outputs."""
    att_w1 = inp["att_w1"]
    att_w2 = inp["att_w2"]

    def unwrap(a, dt=F32):
        a = np.asarray(a, dtype=dt) if dt is not None else np.asarray(a)
        return a.reshape(128, NW, 128).transpose(1, 0, 2).reshape(NLP, 128)

    # global tables (bf16), padded to N rows only
    xtab = np.concatenate(
        [unwrap(p1_results[c]["x_out"])[:NL] for c in range(NCORES)])
    xmtab = np.concatenate(
        [unwrap(np.asarray(p1_results[c]["xm_out"], F32))[:NL]
         for c in range(NCORES)])
    xtab = _bf(xtab)
    xmtab = _bf(xmtab)

    w2op = inp["out_ln_g"][:, None] * inp["out_w2"]
    b2op = inp["out_ln_b"] @ inp["out_w2"] + inp["out_b2"]
    consts = {
        "xtab": xtab, "xmtab": xmtab,
        "iota_part": _f32(np.broadcast_to(np.arange(128)[:, None], (128, 512))),
        "iota_free": _f32(np.broadcast_to(np.arange(128)[None, :], (128, 128))),
        "wc": _bf(att_w1[D:2 * D]), "we_row": _bf(att_w1[2 * D:2 * D + 1]),
        "w2col": _bf(att_w2[:, 0:1]),
        "ident": _bf(np.eye(128)), "ones1": _bf(np.ones((1, 128))),
        "w1o": _bf(inp["out_w1"]), "b1o_row": _bf(inp["out_b1"][None, :]),
        "w2op": _bf(w2op), "b2op_row": _bf(b2op[None, :]),
        "lng_rep": _f32(np.broadcast_to(inp["ln_g"][None, :], (128, 128))),
        "lnb_rep": _f32(np.broadcast_to(inp["ln_b"][None, :], (128, 128))),
        "eps_col": _f32(np.full((128, 1), LN_EPS)),
        "b2_col": _f32(np.full((128, 1), inp2_b2)),
    }
    in_maps = []
    for c in range(NCORES):
        in_maps.append({
            "aloc": np.ascontiguousarray(np.asarray(p1_results[c]["a_out"])),
            "xres": np.ascontiguousarray(np.asarray(p1_results[c]["x_out"])),
            "sel_col": streams["sel_col"][c],
            "idx": streams["idx"][c],
            "lrow_row": streams["lrow_row"][c],
            "ea_row": streams["ea_row"][c],
            **consts,
        })
    return in_maps


def kernel(**inputs):
    inp = {k: np.asarray(v) for k, v in inputs.items()}
    h = _f32(inp["h"])
    weights = (inp["lin_w"], inp["lin_b"], inp["msg_w1"], inp["msg_b1"],
               inp["msg_ln_g"], inp["msg_ln_b"], inp["msg_w2"], inp["msg_b2"],
               inp["att_w1"], inp["att_b1"])
    b2 = float(np.asarray(inp["att_b2"]).ravel()[0])
    plan, streams = _prep_edges(inp["row"], inp["col"], inp["edge_attr"],
                                inp["edge_mask"])

    p1 = _run_phase1(h, weights)

    key = (tuple(plan["WL"]), tuple(plan["WH"]))
    if _cache.get("p2_key") != key:
        _cache["p2"] = _build_phase2(plan)
        _cache["p2_key"] = key
    nc2 = _cache["p2"]
    in_maps = _phase2_inputs(plan, streams, p1, inp, b2)
    res = run_bass_kernel_spmd(nc2, in_maps, core_ids=list(range(NCORES)),
                               trace=TRACE)
    if TRACE:
        LAST_RESULTS["phase2_ns"] = res.exec_time_ns
    out = np.concatenate([
        np.asarray(res.results[c]["out"], F32)
        .reshape(128, NW, 128).transpose(1, 0, 2).reshape(NLP, 128)[:NL]
        for c in range(NCORES)])
    return out.astype(F32)
